# revision 1
# baseline (speedup 1.0000x reference)
"""Trainium2 Bass kernel for nn_DConv2dBlock (deformable conv block).

Pipeline per batch image (batch sharded 2-per-core across 8 cores):
  1. offset = 3x3 conv(x)          [PE, shifted bf16 matmuls, PSUM accumulate]
  2. hat masks for all 81 (k, s) combos via the identity
         vertical weight at shift s = Lambda(dy - s), Lambda(u) = relu(1 - |u|)
     computed packed as [81, N] with ACT (Abs, Relu) + one Pool multiply
  3. masks replicated across the 32 input channels by DMA fan-out from DRAM
  4. product tiles (mask * shifted-x) on DVE feed PE directly; one PSUM
     chain accumulates all 27 (k-group, s) matmuls -> deformable conv output
  5. BN stats via ACT accum_out fused with PSUM evacuation; 2x2 max pooling
     runs inline on the pre-BN activations (maxpool commutes with the BN
     affine since scf >= 0); after the 8-core AllReduce only a tiny
     affine+relu on the pooled maxima remains.

DMAs are spread across the SP / Activation / Pool queues so no single
queue serializes the kernel.

The modulator branch of the reference is dead code and is skipped.
conv bias cancels inside BatchNorm and is skipped.
Requires max|offset| < 1 (checked on host; falls back to a full host
computation in the measure-zero case where it does not hold).
"""

import os
import sys
import numpy as np

for _p in ("/opt/trn_rl_repo",):
    if os.path.isdir(_p) and _p not in sys.path:
        sys.path.insert(0, _p)

B, C, H, W = 16, 32, 128, 128
O = 64
NCORES = 8
BPC = B // NCORES          # batches per core
PADG = 4                   # apron width of the padded image grid
Q = W + 2 * PADG           # padded row length (136)
QQ = Q * Q                 # padded image size
NN = H * W                 # interior pixels (16384)
EPS = 1e-5
NTOT = float(B * NN)
CH = 2048                  # chunk: 16 image rows
MH = 4096                  # mask pipeline chunk
HH = H // 2                # half height (64)
NH = HH * W                # pixels per half (8192)
XKROWS = 20                # rows staged per chunk (16 + 2 apron each side)
XKSZ = XKROWS * Q
KGROUPS = [(0, 4), (4, 4), (8, 1)]   # (kbase, n_k) partition groups
NCH = NN // CH             # chunks per image (8)

_CACHE = {}


def _build_nc(reps=1):
    import concourse.bass as bass
    import concourse.bacc as bacc
    import concourse.mybir as mybir
    from concourse import tile
    from contextlib import ExitStack

    f32 = mybir.dt.float32
    bf16 = mybir.dt.bfloat16
    AF = mybir.ActivationFunctionType

    nc = bacc.Bacc(num_devices=NCORES)
    x_d = nc.dram_tensor("x_sh", [BPC, C, H, W], bf16, kind="ExternalInput")
    woff_d = [
        nc.dram_tensor("woff0", [128, 18], bf16, kind="ExternalInput"),
        nc.dram_tensor("woff1", [128, 18], bf16, kind="ExternalInput"),
        nc.dram_tensor("woff2", [32, 18], bf16, kind="ExternalInput"),
    ]
    wd_d = [
        nc.dram_tensor("wd0", [128, O], bf16, kind="ExternalInput"),
        nc.dram_tensor("wd1", [128, O], bf16, kind="ExternalInput"),
        nc.dram_tensor("wd2", [128, O], bf16, kind="ExternalInput"),
    ]
    offb_d = nc.dram_tensor("offb", [18, 1], f32, kind="ExternalInput")
    gam_d = nc.dram_tensor("gamma", [O, 1], f32, kind="ExternalInput")
    bet_d = nc.dram_tensor("beta", [O, 1], f32, kind="ExternalInput")
    sy_d = nc.dram_tensor("syneg", [81, 1], f32, kind="ExternalInput")
    sx_d = nc.dram_tensor("sxneg", [81, 1], f32, kind="ExternalInput")
    out_d = nc.dram_tensor("out", [BPC, O, H // 2, W // 2], f32,
                           kind="ExternalOutput")

    with tile.TileContext(nc) as tc, ExitStack() as ctx:
        dram = ctx.enter_context(tc.tile_pool(name="dram", bufs=1,
                                              space="DRAM"))
        OFFd = dram.tile([BPC, 18, NN], bf16)
        M81d = dram.tile([BPC, 81, NN], bf16)
        cc_in = dram.tile([O, 2], f32)
        cc_out = dram.tile([O, 2], f32)

        consts = ctx.enter_context(tc.tile_pool(name="consts", bufs=1))
        wof_sb = []
        for g, (kb, ng) in enumerate(KGROUPS):
            t = consts.tile([ng * 32, 18], bf16, tag=f"wof{g}",
                            name=f"wof{g}")
            nc.gpsimd.dma_start(t[:], woff_d[g][:])
            wof_sb.append(t)
        wd_sb = []
        for g in range(3):
            t = consts.tile([128, O], bf16, tag=f"wd{g}", name=f"wd{g}")
            nc.gpsimd.dma_start(t[:], wd_d[g][:])
            wd_sb.append(t)
        offb_sb = consts.tile([18, 1], f32)
        nc.gpsimd.dma_start(offb_sb[:], offb_d[:])
        gam_sb = consts.tile([O, 1], f32)
        nc.gpsimd.dma_start(gam_sb[:], gam_d[:])
        bet_sb = consts.tile([O, 1], f32)
        nc.gpsimd.dma_start(bet_sb[:], bet_d[:])
        sy_sb = consts.tile([81, 1], f32)
        nc.gpsimd.dma_start(sy_sb[:], sy_d[:])
        sx_sb = consts.tile([81, 1], f32)
        nc.gpsimd.dma_start(sx_sb[:], sx_d[:])
        # per (b, chunk) partial-sum cells: col (16b + 2ci + {0:S1, 1:S2})
        accp = consts.tile([O, 4 * NCH], f32)
        s12 = consts.tile([O, 2], f32)
        epsb = consts.tile([O, 1], f32)
        nc.vector.memset(epsb[:], EPS)

        xp_pool = ctx.enter_context(tc.tile_pool(name="xp", bufs=1))
        Xpb = xp_pool.tile([C, QQ], bf16)
        nc.vector.memset(Xpb[:], 0.0)

        psum = ctx.enter_context(tc.tile_pool(name="psum", bufs=2,
                                              space="PSUM"))

        # mask fan-out loads are split 16/27 gpsimd, 11/27 SP (Bresenham)
        for rep in range(reps):
            with tc.tile_pool(name="pooled", bufs=1) as plp:
                pooled = {}
                for b in range(BPC):
                    t = plp.tile([O, NN // 4], bf16, tag=f"pmx{b}",
                                 name=f"pmx{b}")
                    pooled[("pmx", b)] = t
                with tc.tile_pool(name="xk", bufs=2) as xkp, \
                     tc.tile_pool(name="mask", bufs=2) as mp, \
                     tc.tile_pool(name="mchain", bufs=1) as mcp, \
                     tc.tile_pool(name="mrp", bufs=6) as mrp, \
                     tc.tile_pool(name="pp", bufs=8) as ppp, \
                     tc.tile_pool(name="ocp", bufs=2) as ocp, \
                     tc.tile_pool(name="pwp", bufs=1) as pwp, \
                     tc.tile_pool(name="scrp", bufs=3) as scrp, \
                 tc.tile_pool(name="sqp", bufs=1) as sqp:

                    def stage_a(b, ci):
                        """xk staging + offset conv + mask pipeline for chunk ci."""
                        if ci == 0:
                            # load x interior as 4 row strips (apron stays 0) so
                            # early chunks start before the whole image lands
                            for st in range(4):
                                r0s = st * 32
                                xin = Xpb[:, (PADG + r0s) * Q + PADG:
                                          (PADG + r0s) * Q + PADG
                                          + 31 * Q + W]
                                xv = bass.AP(xin.tensor, xin.offset,
                                             [xin.ap[0], [Q, 32], [1, W]])
                                eng = (nc.sync, nc.scalar)[st % 2]
                                eng.dma_start(xv, x_d[b, :, r0s:r0s + 32])
                        row0 = 16 * ci
                        xkbase = (PADG + row0 - 2) * Q
                        c0 = ci * CH
                        xks = []
                        nst = 0
                        for g, (kb, ng) in enumerate(KGROUPS[:2]):
                            xk = xkp.tile([ng * 32, XKSZ], bf16,
                                          tag=f"xk{g}", name=f"xk{g}")
                            for kk in range(ng):
                                k = kb + kk
                                ki, kj = divmod(k, 3)
                                dlt = (ki - 1) * Q + (kj - 1)
                                eng = (nc.scalar, nc.gpsimd,
                                       nc.sync)[nst % 3]
                                nst += 1
                                eng.dma_start(
                                    xk[kk * 32:(kk + 1) * 32, :],
                                    Xpb[:, xkbase + dlt:
                                        xkbase + dlt + XKSZ])
                            xks.append(xk)
                        # offset conv for this chunk; k=8 rides on xk1's
                        # k=7 block shifted one column right
                        oco = 2 * Q + PADG
                        pso = psum.tile([O, CH], f32, tag="ps", name="pso")
                        xb8 = xkbase + (Q + 1) + oco
                        for q4 in range(CH // 512):
                            mms = ((xks[0], oco, wof_sb[0]),
                                   (xks[1], oco, wof_sb[1]),
                                   (Xpb, xb8, wof_sb[2]))
                            for gi, (xkt, oc0, lhs) in enumerate(mms):
                                rhs = xkt[:, oc0 + q4 * 4 * Q:
                                          oc0 + (q4 * 4 + 4) * Q].rearrange(
                                    "p (h q) -> p h q", q=Q)[:, :, 0:W]
                                nc.tensor.matmul(
                                    pso[0:18, q4 * 512:(q4 + 1) * 512],
                                    lhs[:], rhs,
                                    start=(gi == 0), stop=(gi == 2))
                        oc = ocp.tile([18, CH], bf16, tag="oc", name="oc")
                        nc.scalar.activation(oc[:], pso[0:18, :], AF.Identity,
                                             bias=offb_sb[:])
                        nc.scalar.dma_start(OFFd[b, :, c0:c0 + CH], oc[:])
                        # masks for this chunk
                        sl = OFFd[b, 0:1, c0:c0 + CH]
                        dy = mp.tile([81, CH], bf16, tag="dy", name="dy")
                        nc.sync.dma_start(
                            dy[:, :],
                            bass.AP(sl.tensor, sl.offset,
                                    [[2 * NN, 9], [0, 9], [1, CH]]))
                        dx = mp.tile([81, CH], bf16, tag="dx", name="dx")
                        nc.sync.dma_start(
                            dx[:, :],
                            bass.AP(sl.tensor, sl.offset + NN,
                                    [[2 * NN, 9], [0, 9], [1, CH]]))
                        a1 = mcp.tile([81, CH], bf16, tag="a1", name="a1")
                        nc.scalar.activation(a1[:], dy[:], AF.Abs, bias=sy_sb[:])
                        vy = mcp.tile([81, CH], bf16, tag="vy", name="vy")
                        nc.scalar.activation(vy[:], a1[:], AF.Relu,
                                             bias=1.0, scale=-1.0)
                        a2 = mcp.tile([81, CH], bf16, tag="a1", name="a2")
                        nc.scalar.activation(a2[:], dx[:], AF.Abs, bias=sx_sb[:])
                        vx = mcp.tile([81, CH], bf16, tag="vx", name="vx")
                        nc.scalar.activation(vx[:], a2[:], AF.Relu,
                                             bias=1.0, scale=-1.0)
                        m81 = mp.tile([81, CH], bf16, tag="m81", name="m81")
                        nc.vector.tensor_mul(m81[:], vy[:], vx[:])
                        nc.sync.dma_start(M81d[b, :, c0:c0 + CH], m81[:])
                        return xks

                    def stage_b(b, ci, xks):
                        """deform products + one 21-step PSUM chain.

                        k=0..7 contract as two 4k x 32c groups per shift.
                        k=8 needs no own staging: the xk block offsets
                        enumerate the same 3x3 grid as the shift offsets,
                        so its products for shifts 0-3 (4-7) read xk0 (xk1)
                        at one uniform column offset, 4 shifts packed in
                        128 partitions, summed by 4x-replicated weights."""
                        c0 = ci * CH
                        ps = psum.tile([O, CH], f32, tag="ps", name="ps")
                        specs = []
                        for si in range(9):
                            syv, sxv = divmod(si, 3)
                            syv -= 1
                            sxv -= 1
                            o0 = (2 + syv) * Q + PADG + sxv
                            for g in range(2):
                                specs.append((4 * g * 9 + si, 9 * NN, 4,
                                              xks[g], 0, o0, wd_sb[g], 128))
                        v8 = 3 * Q + PADG + 1
                        specs.append((72, NN, 4, xks[0], 0, v8,
                                      wd_sb[2], 128))
                        specs.append((76, NN, 4, xks[1], 0, v8,
                                      wd_sb[2], 128))
                        specs.append((80, NN, 1, xks[1], 64, v8 + 2,
                                      wd_sb[2], 32))
                        nspec = len(specs)
                        for idx, (r0, rstride, nrep, xkt, pb, o0, lhs,
                                  parts) in enumerate(specs):
                            msl = M81d[b, r0:r0 + 1, c0:c0 + CH]
                            fan = bass.AP(msl.tensor, msl.offset,
                                          [[rstride, nrep], [0, 32],
                                           [1, CH]])
                            mr = mrp.tile([128, CH], bf16,
                                          tag="mr", name="mr")
                            up = ((idx + 1) * 12) // nspec > \
                                (idx * 12) // nspec
                            meng = nc.gpsimd if up else nc.sync
                            meng.dma_start(mr[pb:pb + parts, :], fan)
                            pt = ppp.tile([128, CH], bf16,
                                          tag="pt", name="pt")
                            xkv = xkt[pb:pb + parts,
                                      o0:o0 + 16 * Q].rearrange(
                                "p (h q) -> p h q", q=Q)[:, :, 0:W]
                            mv = mr[pb:pb + parts, :].rearrange(
                                "p (h w) -> p h w", w=W)
                            pv = pt[pb:pb + parts, :].rearrange(
                                "p (h w) -> p h w", w=W)
                            nc.vector.tensor_mul(pv, mv, xkv)
                            for q4 in range(CH // 512):
                                nc.tensor.matmul(
                                    ps[:, q4 * 512:(q4 + 1) * 512],
                                    lhs[pb:pb + parts, :],
                                    pt[pb:pb + parts,
                                       q4 * 512:(q4 + 1) * 512],
                                    start=(idx == 0),
                                    stop=(idx == nspec - 1))
                        # evacuate + BN partials for this chunk
                        col = 2 * (NCH * b + ci)
                        scr = scrp.tile([O, CH], bf16, tag="scr", name="scr")
                        nc.scalar.activation(scr[:], ps[:], AF.Identity,
                                             accum_out=accp[:, col:col + 1])
                        sq = sqp.tile([O, CH], bf16, tag="sq", name="sq")
                        nc.scalar.activation(sq[:], scr[:], AF.Square,
                                             accum_out=accp[:, col + 1:col + 2])
                        # inline 2x2 max pooling of the pre-BN activations
                        rv = scr[:, :].rearrange("p (h w) -> p h w", w=W)
                        pw = pwp.tile([O, CH // 2], bf16, tag="pw", name="pw")
                        pwv = pw[:, :].rearrange("p (h w) -> p h w", w=W // 2)
                        nc.vector.tensor_max(pwv, rv[:, :, 0:W:2], rv[:, :, 1:W:2])
                        pw3 = pw[:, :].rearrange("p (h w) -> p h w", w=W // 2)
                        mxs = pooled[("pmx", b)][:, ci * (CH // 4):
                                                 (ci + 1) * (CH // 4)]
                        nc.vector.tensor_max(
                            mxs.rearrange("p (h w) -> p h w", w=W // 2),
                            pw3[:, 0:16:2], pw3[:, 1:16:2])

                    seq = [(b, ci) for b in range(BPC) for ci in range(NCH)]
                    prev = None
                    for step in range(len(seq) + 1):
                        cur = stage_a(*seq[step]) if step < len(seq) else None
                        if prev is not None:
                            stage_b(*seq[step - 1], prev)
                        prev = cur

                with tc.tile_pool(name="fin", bufs=1) as fin:
                    # ---- BN: combine partials, allreduce across cores ----
                    # accp columns: even = S1 cells, odd = S2 cells, 16 of each
                    nc.vector.tensor_add(accp[:, 0:16], accp[:, 0:16], accp[:, 16:32])
                    nc.vector.tensor_add(accp[:, 0:8], accp[:, 0:8], accp[:, 8:16])
                    nc.vector.tensor_add(accp[:, 0:4], accp[:, 0:4], accp[:, 4:8])
                    nc.vector.tensor_add(s12[:, :], accp[:, 0:2], accp[:, 2:4])
                    nc.sync.dma_start(cc_in[:], s12[:])
                    nc.gpsimd.collective_compute(
                        "AllReduce", mybir.AluOpType.add,
                        replica_groups=[list(range(NCORES))],
                        ins=[cc_in.opt()], outs=[cc_out.opt()])

                    s12r = fin.tile([O, 2], f32)
                    nc.sync.dma_start(s12r[:], cc_out[:])
                    mr_ = fin.tile([O, 1], f32, tag="mr_", name="mr_")
                    nc.vector.tensor_scalar_mul(mr_[:], s12r[:, 0:1], 1.0 / NTOT)
                    ex2 = fin.tile([O, 1], f32, tag="ex2", name="ex2")
                    nc.vector.tensor_scalar_mul(ex2[:], s12r[:, 1:2], 1.0 / NTOT)
                    msq = fin.tile([O, 1], f32, tag="msq", name="msq")
                    nc.vector.tensor_mul(msq[:], mr_[:], mr_[:])
                    var = fin.tile([O, 1], f32, tag="var", name="var")
                    nc.vector.tensor_sub(var[:], ex2[:], msq[:])
                    sd = fin.tile([O, 1], f32, tag="sd", name="sd")
                    nc.scalar.activation(sd[:], var[:], AF.Sqrt, bias=epsb[:])
                    inv = fin.tile([O, 1], f32, tag="inv", name="inv")
                    nc.vector.reciprocal(inv[:], sd[:])
                    scf = fin.tile([O, 1], f32, tag="scf", name="scf")
                    nc.vector.tensor_mul(scf[:], gam_sb[:], inv[:])
                    tmp = fin.tile([O, 1], f32, tag="tmp", name="tmp")
                    nc.vector.tensor_mul(tmp[:], mr_[:], scf[:])
                    bif = fin.tile([O, 1], f32, tag="bif", name="bif")
                    nc.vector.tensor_sub(bif[:], bet_sb[:], tmp[:])

                    # ---- affine + relu on pooled maxima + store ----
                    # maxpool commutes with the affine because scf >= 0
                    # (gamma >= 0 checked on host; else host fallback)
                    for b in range(BPC):
                        r1 = fin.tile([O, NN // 4], bf16, tag="r1", name="r1")
                        nc.vector.tensor_scalar(r1[:], pooled[("pmx", b)][:],
                                                scf[:], bif[:],
                                                op0=mybir.AluOpType.mult,
                                                op1=mybir.AluOpType.add)
                        po = fin.tile([O, NN // 4], f32, tag="po", name="po")
                        nc.vector.tensor_scalar_max(po[:], r1[:], 0.0)
                        nc.sync.dma_start(out_d[b], po[:, :])
    nc.compile()
    return nc


def _prep_inputs(x, offset_w, offset_b, conv_w, gamma, beta):
    """Host-side arrangement of weights into the layouts the kernel wants."""
    import ml_dtypes
    wofs = []
    for kb, ng in KGROUPS:
        blocks = []
        for kk in range(ng):
            ki, kj = divmod(kb + kk, 3)
            blocks.append(offset_w[:, :, ki, kj].T)    # [C, 18]
        wofs.append(np.ascontiguousarray(
            np.concatenate(blocks, axis=0)).astype(ml_dtypes.bfloat16))
    wds = []
    for kb, ng in KGROUPS:
        blocks = []
        for kk in range(ng):
            ki, kj = divmod(kb + kk, 3)
            blocks.append(conv_w[:, :, ki, kj].T)      # [C, O]
        wds.append(np.ascontiguousarray(
            np.concatenate(blocks, axis=0)).astype(ml_dtypes.bfloat16))
    # k=8 weights replicated 4x: the matmul contraction sums the 4 shifts
    # packed into the 128 partitions of its product tiles
    wds[2] = np.ascontiguousarray(np.tile(wds[2], (4, 1)))
    syneg = np.zeros((81, 1), np.float32)
    sxneg = np.zeros((81, 1), np.float32)
    for k in range(9):
        for si in range(9):
            sy, sx = divmod(si, 3)
            syneg[k * 9 + si, 0] = -(sy - 1)
            sxneg[k * 9 + si, 0] = -(sx - 1)
    base = dict(
        woff0=wofs[0], woff1=wofs[1], woff2=wofs[2],
        wd0=wds[0], wd1=wds[1], wd2=wds[2],
        offb=offset_b.reshape(18, 1).astype(np.float32),
        gamma=gamma.reshape(O, 1).astype(np.float32),
        beta=beta.reshape(O, 1).astype(np.float32),
        syneg=syneg, sxneg=sxneg,
    )
    in_maps = []
    for ci in range(NCORES):
        m = dict(base)
        m["x_sh"] = np.ascontiguousarray(
            x[ci * BPC:(ci + 1) * BPC]).astype(ml_dtypes.bfloat16)
        in_maps.append(m)
    return in_maps


def _host_offsets(x, offset_w, offset_b):
    """offset = conv3x3(x, offset_w) + offset_b on host (|off|<1 check)."""
    xpad = np.pad(x, ((0, 0), (0, 0), (1, 1), (1, 1)))
    win = np.lib.stride_tricks.sliding_window_view(xpad, (3, 3), axis=(2, 3))
    cols = win.transpose(0, 2, 3, 1, 4, 5).reshape(B, NN, C * 9)
    w2 = offset_w.reshape(18, C * 9)
    off = cols @ w2.T.astype(np.float32)
    return off.reshape(B, H, W, 18).transpose(0, 3, 1, 2) + \
        offset_b.reshape(1, 18, 1, 1)


def _host_reference(x, offset_w, offset_b, conv_w, conv_b, gamma, beta):
    """Full numpy fallback (used only if some |offset| >= 1)."""
    off = _host_offsets(x, offset_w, offset_b).reshape(B, 9, 2, H, W)
    ki, kj = np.meshgrid(np.arange(3), np.arange(3), indexing="ij")
    base_y = (np.arange(H)[None, :, None] - 1 +
              ki.reshape(9)[:, None, None]).astype(np.float32)
    base_x = (np.arange(W)[None, None, :] - 1 +
              kj.reshape(9)[:, None, None]).astype(np.float32)
    py = off[:, :, 0] + base_y[None]
    px = off[:, :, 1] + base_x[None]
    y0 = np.floor(py).astype(np.int64)
    x0 = np.floor(px).astype(np.int64)
    wy = py - y0
    wx = px - x0
    bidx = np.arange(B)[:, None, None, None]

    def gather(iy, ix):
        valid = (iy >= 0) & (iy < H) & (ix >= 0) & (ix < W)
        v = x[bidx, :, np.clip(iy, 0, H - 1), np.clip(ix, 0, W - 1)]
        return np.where(valid[..., None], v, 0.0)

    val = (gather(y0, x0) * ((1 - wy) * (1 - wx))[..., None]
           + gather(y0, x0 + 1) * ((1 - wy) * wx)[..., None]
           + gather(y0 + 1, x0) * (wy * (1 - wx))[..., None]
           + gather(y0 + 1, x0 + 1) * (wy * wx)[..., None])
    out = np.einsum("bkhwc,ock->bohw", val, conv_w.reshape(O, C, 9),
                    optimize=True) + conv_b[None, :, None, None]
    m = out.mean(axis=(0, 2, 3), keepdims=True)
    v = out.var(axis=(0, 2, 3), keepdims=True)
    out = (out - m) / np.sqrt(v + EPS) * gamma[None, :, None, None] + \
        beta[None, :, None, None]
    out = np.maximum(out, 0.0)
    out = out.reshape(B, O, H // 2, 2, W // 2, 2).max(axis=(3, 5))
    return out.astype(np.float32)


def _get_nc(reps=1):
    key = ("nc", reps)
    if key not in _CACHE:
        _CACHE[key] = _build_nc(reps)
    return _CACHE[key]


def _run_device(in_maps, trace=False):
    from concourse import bass_utils
    nc = _get_nc()
    return bass_utils.run_bass_kernel_spmd(
        nc, in_maps, core_ids=list(range(NCORES)), trace=trace)


def kernel(x, offset_w, offset_b, mod_w, mod_b, conv_w, conv_b, gamma, beta,
           _trace=False, _return_results=False):
    x = np.asarray(x, np.float32)
    offset_w = np.asarray(offset_w, np.float32)
    offset_b = np.asarray(offset_b, np.float32)
    conv_w = np.asarray(conv_w, np.float32)
    conv_b = np.asarray(conv_b, np.float32)
    gamma = np.asarray(gamma, np.float32)
    beta = np.asarray(beta, np.float32)

    off = _host_offsets(x, offset_w, offset_b)
    if np.max(np.abs(off)) >= 0.999999 or np.min(gamma) < 0.0:
        return _host_reference(x, offset_w, offset_b, conv_w, conv_b,
                               gamma, beta)

    in_maps = _prep_inputs(x, offset_w, offset_b, conv_w, gamma, beta)
    res = _run_device(in_maps, trace=False)
    out = np.concatenate([res.results[i]["out"] for i in range(NCORES)],
                         axis=0)
    out = np.ascontiguousarray(out).astype(np.float32)
    if _return_results:
        return out, res
    return out



# revision 8
# speedup vs baseline: 1.0212x; 1.0212x over previous
"""Trainium2 Bass kernel for nn_DConv2dBlock (deformable conv block).

Pixel-major formulation (batch sharded 2 images per core across 8 cores):
  1. offset = 3x3 conv(x): PE PSUM chain of 9 shifted matmuls per chunk
     (rhs = shifted views of a zero-padded c-major image, no staging DMA).
  2. offsets permuted to pixel-major [y, (plane, x)]; triangle masks
     Lambda(dy - s) = relu(1 - |dy - s|) built by ACT; the 81 (sy, k, sx)
     mask planes m81[y, (sy,k,sx,x)] = vy * vx via 3 DVE ops per image.
  3. products in pixel-major [y, (c, x)]: for each (k,s) one DVE op
       p = m81-plane (broadcast over c via stride-0 AP) * XT-slice
     where XT[y, (dy+2, c, xhat)] holds 5 row-shifted copies of the
     x-padded image, so both shift axes are free-dim offsets and no mask
     fan-out DMA exists at all (the channel broadcast happens inside the
     DVE operand read).
  4. per (k, img): val_k = sum of 9 products; 5 adds on DVE, 3 on gpsimd.
  5. val_k dumped to DRAM (contiguous); re-read per chunk with a
     (c, y, x) gather into channel-major [(k,c), CH] tiles; PE contracts
     all 288 (k,c) rows in a 3-matmul PSUM chain per chunk.
  6. BN stats via ACT accum_out on PSUM evacuation; 2x2 maxpool inline on
     pre-BN activations (commutes with the affine since scf >= 0); 8-core
     AllReduce of (S1, S2); tiny affine+relu on pooled maxima.

The modulator branch of the reference is dead code and skipped.
conv bias cancels inside BatchNorm and is skipped.
Requires max|offset| < 1 (checked on host; falls back to a full host
computation in the measure-zero case where it does not hold).
"""

import os
import sys
import numpy as np

for _p in ("/opt/trn_rl_repo",):
    if os.path.isdir(_p) and _p not in sys.path:
        sys.path.insert(0, _p)

B, C, H, W = 16, 32, 128, 128
O = 64
NCORES = 8
BPC = B // NCORES          # images per core
NN = H * W                 # pixels per image (16384)
EPS = 1e-5
NTOT = float(B * NN)
CH = 2048                  # chunk: 16 image rows
NCH = NN // CH             # chunks per image (8)
XH = W + 4                 # padded row width for XT (132)
QW = W + 2                 # padded cols in c-major image (130)
CW = C * W                 # free size of a (c, x) plane (4096)
KGROUPS = [(0, 4), (4, 4), (8, 1)]

_CACHE = {}


def _build_nc(reps=1):
    import concourse.bass as bass
    import concourse.bacc as bacc
    import concourse.mybir as mybir
    from concourse import tile
    from contextlib import ExitStack

    f32 = mybir.dt.float32
    bf16 = mybir.dt.bfloat16
    AF = mybir.ActivationFunctionType
    A = mybir.AluOpType

    nc = bacc.Bacc(num_devices=NCORES)
    x_d = nc.dram_tensor("x_sh", [BPC, C, H, W], bf16, kind="ExternalInput")
    woff_d = nc.dram_tensor("woff", [9, C, 18], bf16, kind="ExternalInput")
    wd_d = [
        nc.dram_tensor("wd0", [128, O], bf16, kind="ExternalInput"),
        nc.dram_tensor("wd1", [128, O], bf16, kind="ExternalInput"),
        nc.dram_tensor("wd2", [32, O], bf16, kind="ExternalInput"),
    ]
    offb_d = nc.dram_tensor("offb", [18, 1], f32, kind="ExternalInput")
    gam_d = nc.dram_tensor("gamma", [O, 1], f32, kind="ExternalInput")
    bet_d = nc.dram_tensor("beta", [O, 1], f32, kind="ExternalInput")
    out_d = nc.dram_tensor("out", [BPC, O, H // 2, W // 2], f32,
                           kind="ExternalOutput")

    with tile.TileContext(nc) as tc, ExitStack() as ctx:
        dram = ctx.enter_context(tc.tile_pool(name="dram", bufs=1,
                                              space="DRAM"))
        OFFd = dram.tile([BPC, 18, NN], bf16)
        VTd = dram.tile([BPC, 9, H, CW], bf16)      # pixel-major val_k
        PLd = dram.tile([BPC, O, NN // 4], bf16)    # pooled maxima
        cc_in = dram.tile([O, 2], f32)
        cc_out = dram.tile([O, 2], f32)

        consts = ctx.enter_context(tc.tile_pool(name="consts", bufs=1))
        wof_sb = consts.tile([C, 9 * 18], bf16)
        nc.sync.dma_start(
            wof_sb[:],
            bass.AP(woff_d[:].tensor, 0, [[18, C], [C * 18, 9], [1, 18]]))
        wd_sb = []
        for g in range(3):
            t = consts.tile([wd_d[g].shape[0], O], bf16, tag=f"wd{g}",
                            name=f"wd{g}")
            nc.sync.dma_start(t[:], wd_d[g][:])
            wd_sb.append(t)
        offb_sb = consts.tile([18, 1], f32)
        nc.sync.dma_start(offb_sb[:], offb_d[:])
        gam_sb = consts.tile([O, 1], f32)
        nc.sync.dma_start(gam_sb[:], gam_d[:])
        bet_sb = consts.tile([O, 1], f32)
        nc.sync.dma_start(bet_sb[:], bet_d[:])
        accp = consts.tile([O, 4 * NCH], f32)
        epsb = consts.tile([O, 1], f32)
        nc.vector.memset(epsb[:], EPS)
        sbias = []
        for s in range(3):
            t = consts.tile([128, 1], f32, tag=f"sb{s}", name=f"sb{s}")
            nc.vector.memset(t[:], float(-(s - 1)))
            sbias.append(t)

        # persistent padded images; edges zeroed once, interiors rewritten
        xp_pool = ctx.enter_context(tc.tile_pool(name="xp", bufs=1))
        Xpb = xp_pool.tile([C, (H + 2) * QW], bf16)  # c-major padded image
        nc.vector.memset(Xpb[:], 0.0)
        XT = xp_pool.tile([128, 5 * C * XH], bf16)   # 5 row-shifted copies
        nc.vector.memset(XT[:], 0.0)

        psum = ctx.enter_context(tc.tile_pool(name="psum", bufs=2,
                                              space="PSUM"))

        def v3(ap):
            return ap.rearrange("p (c x) -> p c x", x=W)

        for rep in range(reps):
            with tc.tile_pool(name="offp", bufs=1) as offp, \
                 tc.tile_pool(name="mskp", bufs=1) as mskp, \
                 tc.tile_pool(name="plp", bufs=4) as plp, \
                 tc.tile_pool(name="acp", bufs=1) as acp, \
                 tc.tile_pool(name="vcp", bufs=2) as vcp, \
                 tc.tile_pool(name="ocp", bufs=1) as ocp, \
                 tc.tile_pool(name="evp", bufs=2) as evp, \
                 tc.tile_pool(name="evq", bufs=1) as evq, \
                 tc.tile_pool(name="fin", bufs=1) as fin:

                def load_images(b):
                    """Fill Xpb (c-major padded) + XT (5 shifted copies)."""
                    for st in range(4):
                        r0 = st * 32
                        xin = Xpb[:, (1 + r0) * QW + 1:
                                  (1 + r0) * QW + 1 + 31 * QW + W]
                        xv = bass.AP(xin.tensor, xin.offset,
                                     [xin.ap[0], [QW, 32], [1, W]])
                        eng = (nc.sync, nc.scalar)[st % 2]
                        eng.dma_start(xv, x_d[b, :, r0:r0 + 32])
                    for d in range(5):
                        dy = d - 2
                        y0 = max(0, -dy)
                        ny = H - abs(dy)
                        xo = XT[y0:y0 + ny,
                                d * C * XH + 2:d * C * XH + 2
                                + (C - 1) * XH + W]
                        xov = bass.AP(xo.tensor, xo.offset,
                                      [xo.ap[0], [XH, C], [1, W]])
                        src = x_d[b, :, y0 + dy:y0 + dy + ny, :]
                        sv = bass.AP(src.tensor, src.offset,
                                     [[W, ny], [H * W, C], [1, W]])
                        eng = (nc.sync, nc.scalar)[d % 2]
                        eng.dma_start(xov, sv)

                def offconv(b):
                    """3x3 conv -> OFFd[b]: PSUM chain of 9 shifted mms."""
                    for ci in range(NCH):
                        row0 = 16 * ci
                        pso = psum.tile([O, CH], f32, tag="ps", name="pso")
                        base = (1 + row0) * QW + 1
                        for k in range(9):
                            ki, kj = divmod(k, 3)
                            dlt = (ki - 1) * QW + (kj - 1)
                            sl = Xpb[:, base + dlt:base + dlt
                                     + 15 * QW + W]
                            for q4 in range(CH // 512):
                                rhs = bass.AP(
                                    sl.tensor, sl.offset + q4 * 4 * QW,
                                    [sl.ap[0], [QW, 4], [1, W]])
                                nc.tensor.matmul(
                                    pso[0:18, q4 * 512:(q4 + 1) * 512],
                                    wof_sb[:, k * 18:(k + 1) * 18], rhs,
                                    start=(k == 0), stop=(k == 8))
                        oc = ocp.tile([18, CH], bf16, tag="oc", name="oc")
                        nc.scalar.activation(oc[:], pso[0:18, :], AF.Identity,
                                             bias=offb_sb[:])
                        nc.scalar.dma_start(
                            OFFd[b, :, ci * CH:(ci + 1) * CH], oc[:])

                def masks(b):
                    """offT -> vy/vx -> m81[y, (sy, k, sx, x)]."""
                    offT = offp.tile([128, 18 * W], bf16, tag="offT",
                                     name="offT")
                    src = OFFd[b]
                    nc.sync.dma_start(
                        offT[:],
                        bass.AP(src.tensor, src.offset,
                                [[W, 128], [NN, 18], [1, W]]))
                    vy = mskp.tile([128, 27 * W], bf16, tag="vy", name="vy")
                    vx = mskp.tile([128, 27 * W], bf16, tag="vx", name="vx")
                    tmp = mskp.tile([128, 9 * W], bf16, tag="tmp",
                                    name="tmp")
                    ov = offT[:]
                    for ax, vt in ((0, vy), (1, vx)):
                        dsl = bass.AP(ov.tensor, ov.offset + ax * W,
                                      [ov.ap[0], [2 * W, 9], [1, W]])
                        for s in range(3):
                            nc.scalar.activation(tmp[:], dsl, AF.Abs,
                                                 bias=sbias[s][:])
                            nc.scalar.activation(
                                vt[:, s * 9 * W:(s + 1) * 9 * W], tmp[:],
                                AF.Relu, bias=1.0, scale=-1.0)
                    m81 = mskp.tile([128, 81 * W], bf16, tag="m81",
                                    name="m81")
                    vyv = vy[:]
                    vxv = vx[:]
                    for sy in range(3):
                        # out [y, (9k, (3sx, x))] = vy[sy-block k] bcast sx
                        #                         * vx[(k, sx)]
                        mo = m81[:, sy * 27 * W:(sy + 1) * 27 * W]
                        mov = bass.AP(mo.tensor, mo.offset,
                                      [mo.ap[0], [3 * W, 9], [1, 3 * W]])
                        in0 = bass.AP(vyv.tensor,
                                      vyv.offset + sy * 9 * W,
                                      [vyv.ap[0], [W, 9], [0, 3], [1, W]])
                        in1 = bass.AP(vxv.tensor, vxv.offset,
                                      [vxv.ap[0], [W, 9], [9 * W, 3],
                                       [1, W]])
                        nc.vector.tensor_tensor(mov, in0, in1, A.mult)
                    return m81

                def deform_k(b, k, m81):
                    """val_k[y, (c,x)] = sum_s m81-plane * XT-slice."""
                    ki, kj = divmod(k, 3)
                    planes = []
                    accA = acp.tile([128, CW], bf16, tag="accA",
                                    name="accA")
                    accB = acp.tile([128, CW], bf16, tag="accB",
                                    name="accB")
                    m81v = m81[:]
                    xtv = XT[:]
                    for si in range(9):
                        sy, sx = divmod(si, 3)
                        d = ki + sy           # 0..4 row-shift version
                        dx = kj + sx          # 0..4 col offset in xhat
                        moff = ((sy * 9 + k) * 3 + sx) * W
                        min1 = bass.AP(m81v.tensor, m81v.offset + moff,
                                       [m81v.ap[0], [0, C], [1, W]])
                        xin0 = bass.AP(xtv.tensor,
                                       xtv.offset + d * C * XH + dx,
                                       [xtv.ap[0], [XH, C], [1, W]])
                        pt = plp.tile([128, CW], bf16, tag="pt",
                                      name=f"pt{si}")
                        nc.vector.tensor_tensor(v3(pt[:]), xin0, min1,
                                                A.mult)
                        planes.append(pt)
                        if si == 1:
                            nc.vector.tensor_add(v3(accA[:]),
                                                 v3(planes[0][:]),
                                                 v3(planes[1][:]))
                        elif 2 <= si <= 5:
                            nc.vector.tensor_add(v3(accA[:]), v3(accA[:]),
                                                 v3(planes[si][:]))
                        elif si == 7:
                            nc.gpsimd.tensor_add(v3(accB[:]),
                                                 v3(planes[6][:]),
                                                 v3(planes[7][:]))
                        elif si == 8:
                            nc.gpsimd.tensor_add(v3(accB[:]), v3(accB[:]),
                                                 v3(planes[8][:]))
                    vt = plp.tile([128, CW], bf16, tag="pt", name="vtj")
                    nc.vector.tensor_add(v3(vt[:]), v3(accA[:]),
                                         v3(accB[:]))
                    # dump to DRAM contiguously (cheap big-run DMA)
                    eng = (nc.sync, nc.scalar)[k % 2]
                    eng.dma_start(VTd[b, k], vt[:])

                def final_chunk(b, ci, pooled_sl):
                    # gather c-major val tiles for this chunk from VTd
                    vals = []
                    for g, (kb, ng) in enumerate(KGROUPS):
                        vtile = vcp.tile([ng * C, CH], bf16, tag=f"val{g}",
                                         name=f"val{g}")
                        src = VTd[b]
                        for kk in range(ng):
                            inap = bass.AP(
                                src.tensor,
                                src.offset + (kb + kk) * H * CW
                                + 16 * ci * CW,
                                [[W, C], [CW, 16], [1, W]])
                            ov = vtile[kk * C:(kk + 1) * C, :]
                            outap = bass.AP(ov.tensor, ov.offset,
                                            [ov.ap[0], [W, 16], [1, W]])
                            eng = (nc.sync, nc.scalar)[(ci + kb + kk) % 2]
                            eng.dma_start(outap, inap)
                        vals.append(vtile)
                    ps = psum.tile([O, CH], f32, tag="ps", name="ps")
                    for g in range(3):
                        for q4 in range(CH // 512):
                            nc.tensor.matmul(
                                ps[:, q4 * 512:(q4 + 1) * 512],
                                wd_sb[g][:],
                                vals[g][:, q4 * 512:(q4 + 1) * 512],
                                start=(g == 0), stop=(g == 2))
                    col = 2 * (NCH * b + ci)
                    scr = evp.tile([O, CH], bf16, tag="scr", name="scr")
                    nc.scalar.activation(scr[:], ps[:], AF.Identity,
                                         accum_out=accp[:, col:col + 1])
                    rv = scr[:, :].rearrange("p (h w) -> p h w", w=W)
                    pw = evq.tile([O, CH // 2], bf16, tag="pw", name="pw")
                    pwv = pw[:, :].rearrange("p (h w) -> p h w", w=W // 2)
                    nc.vector.tensor_max(pwv, rv[:, :, 0:W:2],
                                         rv[:, :, 1:W:2])
                    pw3 = pw[:, :].rearrange("p (h w) -> p h w", w=W // 2)
                    mxs = evp.tile([O, CH // 4], bf16, tag="mxs",
                                   name="mxs")
                    nc.vector.tensor_max(
                        mxs[:].rearrange("p (h w) -> p h w", w=W // 2),
                        pw3[:, 0:16:2], pw3[:, 1:16:2])
                    nc.scalar.activation(scr[:], scr[:], AF.Square,
                                         accum_out=accp[:, col + 1:col + 2])
                    nc.sync.dma_start(pooled_sl, mxs[:])

                # ---------------- main schedule ----------------
                for b in range(BPC):
                    load_images(b)
                    offconv(b)
                    m81 = masks(b)
                    for k in range(9):
                        deform_k(b, k, m81)
                    for ci in range(NCH):
                        final_chunk(b, ci,
                                    PLd[b, :, ci * (CH // 4):
                                        (ci + 1) * (CH // 4)])

                # ---- BN: combine partials, allreduce across cores ----
                s12 = fin.tile([O, 2], f32, tag="s12", name="s12")
                nc.vector.tensor_add(accp[:, 0:16], accp[:, 0:16],
                                     accp[:, 16:32])
                nc.vector.tensor_add(accp[:, 0:8], accp[:, 0:8],
                                     accp[:, 8:16])
                nc.vector.tensor_add(accp[:, 0:4], accp[:, 0:4],
                                     accp[:, 4:8])
                nc.vector.tensor_add(s12[:, :], accp[:, 0:2], accp[:, 2:4])
                nc.sync.dma_start(cc_in[:], s12[:])
                nc.gpsimd.collective_compute(
                    "AllReduce", mybir.AluOpType.add,
                    replica_groups=[list(range(NCORES))],
                    ins=[cc_in.opt()], outs=[cc_out.opt()])

                s12r = fin.tile([O, 2], f32, tag="s12r", name="s12r")
                nc.sync.dma_start(s12r[:], cc_out[:])
                mr_ = fin.tile([O, 1], f32, tag="mr_", name="mr_")
                nc.vector.tensor_scalar_mul(mr_[:], s12r[:, 0:1],
                                            1.0 / NTOT)
                ex2 = fin.tile([O, 1], f32, tag="ex2", name="ex2")
                nc.vector.tensor_scalar_mul(ex2[:], s12r[:, 1:2],
                                            1.0 / NTOT)
                msq = fin.tile([O, 1], f32, tag="msq", name="msq")
                nc.vector.tensor_mul(msq[:], mr_[:], mr_[:])
                var = fin.tile([O, 1], f32, tag="var", name="var")
                nc.vector.tensor_sub(var[:], ex2[:], msq[:])
                sd = fin.tile([O, 1], f32, tag="sd", name="sd")
                nc.scalar.activation(sd[:], var[:], AF.Sqrt, bias=epsb[:])
                inv = fin.tile([O, 1], f32, tag="inv", name="inv")
                nc.vector.reciprocal(inv[:], sd[:])
                scf = fin.tile([O, 1], f32, tag="scf", name="scf")
                nc.vector.tensor_mul(scf[:], gam_sb[:], inv[:])
                tmp2 = fin.tile([O, 1], f32, tag="tmp2", name="tmp2")
                nc.vector.tensor_mul(tmp2[:], mr_[:], scf[:])
                bif = fin.tile([O, 1], f32, tag="bif", name="bif")
                nc.vector.tensor_sub(bif[:], bet_sb[:], tmp2[:])

                # ---- affine + relu on pooled maxima + store ----
                for b in range(BPC):
                    for q in range(16):
                        sl = PLd[b, :, q * 256:(q + 1) * 256]
                        plb = fin.tile([O, 256], bf16, tag="plb",
                                       name="plb")
                        nc.sync.dma_start(plb[:], sl)
                        r1 = fin.tile([O, 256], bf16, tag="r1", name="r1")
                        nc.vector.tensor_scalar(
                            r1[:], plb[:], scf[:], bif[:],
                            op0=mybir.AluOpType.mult,
                            op1=mybir.AluOpType.add)
                        po = fin.tile([O, 256], f32, tag="po", name="po")
                        nc.vector.tensor_scalar_max(po[:], r1[:], 0.0)
                        od = out_d[b]
                        nc.sync.dma_start(
                            bass.AP(od.tensor, od.offset + q * 256,
                                    [[NN // 4, O], [1, 256]]),
                            po[:, :])
    nc.compile()
    return nc


def _prep_inputs(x, offset_w, offset_b, conv_w, gamma, beta):
    """Host-side arrangement of weights into the layouts the kernel wants."""
    import ml_dtypes
    woff = np.zeros((9, C, 18), np.float32)
    for k in range(9):
        ki, kj = divmod(k, 3)
        woff[k] = offset_w[:, :, ki, kj].T
    wds = []
    for kb, ng in KGROUPS:
        blocks = []
        for kk in range(ng):
            ki, kj = divmod(kb + kk, 3)
            blocks.append(conv_w[:, :, ki, kj].T)      # [C, O]
        wds.append(np.ascontiguousarray(
            np.concatenate(blocks, axis=0)).astype(ml_dtypes.bfloat16))
    base = dict(
        woff=np.ascontiguousarray(woff).astype(ml_dtypes.bfloat16),
        wd0=wds[0], wd1=wds[1], wd2=wds[2],
        offb=offset_b.reshape(18, 1).astype(np.float32),
        gamma=gamma.reshape(O, 1).astype(np.float32),
        beta=beta.reshape(O, 1).astype(np.float32),
    )
    in_maps = []
    for ci in range(NCORES):
        m = dict(base)
        m["x_sh"] = np.ascontiguousarray(
            x[ci * BPC:(ci + 1) * BPC]).astype(ml_dtypes.bfloat16)
        in_maps.append(m)
    return in_maps


def _host_offsets(x, offset_w, offset_b):
    """offset = conv3x3(x, offset_w) + offset_b on host (|off|<1 check)."""
    xpad = np.pad(x, ((0, 0), (0, 0), (1, 1), (1, 1)))
    win = np.lib.stride_tricks.sliding_window_view(xpad, (3, 3), axis=(2, 3))
    cols = win.transpose(0, 2, 3, 1, 4, 5).reshape(B, NN, C * 9)
    w2 = offset_w.reshape(18, C * 9)
    off = cols @ w2.T.astype(np.float32)
    return off.reshape(B, H, W, 18).transpose(0, 3, 1, 2) + \
        offset_b.reshape(1, 18, 1, 1)


def _host_reference(x, offset_w, offset_b, conv_w, conv_b, gamma, beta):
    """Full numpy fallback (used only if some |offset| >= 1)."""
    off = _host_offsets(x, offset_w, offset_b).reshape(B, 9, 2, H, W)
    ki, kj = np.meshgrid(np.arange(3), np.arange(3), indexing="ij")
    base_y = (np.arange(H)[None, :, None] - 1 +
              ki.reshape(9)[:, None, None]).astype(np.float32)
    base_x = (np.arange(W)[None, None, :] - 1 +
              kj.reshape(9)[:, None, None]).astype(np.float32)
    py = off[:, :, 0] + base_y[None]
    px = off[:, :, 1] + base_x[None]
    y0 = np.floor(py).astype(np.int64)
    x0 = np.floor(px).astype(np.int64)
    wy = py - y0
    wx = px - x0
    bidx = np.arange(B)[:, None, None, None]

    def gather(iy, ix):
        valid = (iy >= 0) & (iy < H) & (ix >= 0) & (ix < W)
        v = x[bidx, :, np.clip(iy, 0, H - 1), np.clip(ix, 0, W - 1)]
        return np.where(valid[..., None], v, 0.0)

    val = (gather(y0, x0) * ((1 - wy) * (1 - wx))[..., None]
           + gather(y0, x0 + 1) * ((1 - wy) * wx)[..., None]
           + gather(y0 + 1, x0) * (wy * (1 - wx))[..., None]
           + gather(y0 + 1, x0 + 1) * (wy * wx)[..., None])
    out = np.einsum("bkhwc,ock->bohw", val, conv_w.reshape(O, C, 9),
                    optimize=True) + conv_b[None, :, None, None]
    m = out.mean(axis=(0, 2, 3), keepdims=True)
    v = out.var(axis=(0, 2, 3), keepdims=True)
    out = (out - m) / np.sqrt(v + EPS) * gamma[None, :, None, None] + \
        beta[None, :, None, None]
    out = np.maximum(out, 0.0)
    out = out.reshape(B, O, H // 2, 2, W // 2, 2).max(axis=(3, 5))
    return out.astype(np.float32)


def _get_nc(reps=1):
    key = ("nc", reps)
    if key not in _CACHE:
        _CACHE[key] = _build_nc(reps)
    return _CACHE[key]


def _run_device(in_maps, trace=False):
    from concourse import bass_utils
    nc = _get_nc()
    return bass_utils.run_bass_kernel_spmd(
        nc, in_maps, core_ids=list(range(NCORES)), trace=trace)


def kernel(x, offset_w, offset_b, mod_w, mod_b, conv_w, conv_b, gamma, beta,
           _trace=False, _return_results=False):
    x = np.asarray(x, np.float32)
    offset_w = np.asarray(offset_w, np.float32)
    offset_b = np.asarray(offset_b, np.float32)
    conv_w = np.asarray(conv_w, np.float32)
    conv_b = np.asarray(conv_b, np.float32)
    gamma = np.asarray(gamma, np.float32)
    beta = np.asarray(beta, np.float32)

    off = _host_offsets(x, offset_w, offset_b)
    if np.max(np.abs(off)) >= 0.999999 or np.min(gamma) < 0.0:
        return _host_reference(x, offset_w, offset_b, conv_w, conv_b,
                               gamma, beta)

    in_maps = _prep_inputs(x, offset_w, offset_b, conv_w, gamma, beta)
    res = _run_device(in_maps, trace=False)
    out = np.concatenate([res.results[i]["out"] for i in range(NCORES)],
                         axis=0)
    out = np.ascontiguousarray(out).astype(np.float32)
    if _return_results:
        return out, res
    return out


# revision 10
# speedup vs baseline: 1.2325x; 1.2069x over previous
"""Trainium2 Bass kernel for nn_DConv2dBlock (deformable conv block).

Pixel-major formulation (batch sharded 2 images per core across 8 cores):
  1. offset = 3x3 conv(x): PE PSUM chain of 9 shifted matmuls per chunk
     (rhs = shifted views of a zero-padded c-major image, no staging DMA).
  2. offsets permuted to pixel-major [y, (plane, x)]; triangle masks
     Lambda(dy - s) = relu(1 - |dy - s|) built by ACT; the 81 (sy, k, sx)
     mask planes m81[y, (sy,k,sx,x)] = vy * vx via 3 DVE ops per image.
  3. products in pixel-major [y, (c, x)]: for each (k,s) one DVE op
       p = m81-plane (broadcast over c via stride-0 AP) * XT-slice
     where XT[y, (dy+2, c, xhat)] holds 5 row-shifted copies of the
     x-padded image, so both shift axes are free-dim offsets and no mask
     fan-out DMA exists at all (the channel broadcast happens inside the
     DVE operand read).
  4. per (k, img): val_k = sum of 9 products; 5 adds on DVE, 3 on gpsimd.
  5. val_k dumped to DRAM (contiguous); re-read per chunk with a
     (c, y, x) gather into channel-major [(k,c), CH] tiles; PE contracts
     all 288 (k,c) rows in a 3-matmul PSUM chain per chunk.
  6. BN stats via ACT accum_out on PSUM evacuation; 2x2 maxpool inline on
     pre-BN activations (commutes with the affine since scf >= 0); 8-core
     AllReduce of (S1, S2); tiny affine+relu on pooled maxima.

The modulator branch of the reference is dead code and skipped.
conv bias cancels inside BatchNorm and is skipped.
Requires max|offset| < 1 (checked on host; falls back to a full host
computation in the measure-zero case where it does not hold).
"""

import os
import sys
import numpy as np

for _p in ("/opt/trn_rl_repo",):
    if os.path.isdir(_p) and _p not in sys.path:
        sys.path.insert(0, _p)

B, C, H, W = 16, 32, 128, 128
O = 64
NCORES = 8
BPC = B // NCORES          # images per core
NN = H * W                 # pixels per image (16384)
EPS = 1e-5
NTOT = float(B * NN)
CH = 2048                  # chunk: 16 image rows
NCH = NN // CH             # chunks per image (8)
XH = W + 4                 # padded row width for XT (132)
QW = W + 2                 # padded cols in c-major image (130)
CW = C * W                 # free size of a (c, x) plane (4096)
KGROUPS = [(0, 4), (4, 4), (8, 1)]

_CACHE = {}


def _build_nc(reps=1):
    import concourse.bass as bass
    import concourse.bacc as bacc
    import concourse.mybir as mybir
    from concourse import tile
    from contextlib import ExitStack

    f32 = mybir.dt.float32
    bf16 = mybir.dt.bfloat16
    AF = mybir.ActivationFunctionType
    A = mybir.AluOpType

    nc = bacc.Bacc(num_devices=NCORES)
    x_d = nc.dram_tensor("x_sh", [BPC, C, H, W], bf16, kind="ExternalInput")
    woff_d = nc.dram_tensor("woff", [9, C, 18], bf16, kind="ExternalInput")
    wd_d = [
        nc.dram_tensor("wd0", [128, O], bf16, kind="ExternalInput"),
        nc.dram_tensor("wd1", [128, O], bf16, kind="ExternalInput"),
        nc.dram_tensor("wd2", [32, O], bf16, kind="ExternalInput"),
    ]
    offb_d = nc.dram_tensor("offb", [18, 1], f32, kind="ExternalInput")
    gam_d = nc.dram_tensor("gamma", [O, 1], f32, kind="ExternalInput")
    bet_d = nc.dram_tensor("beta", [O, 1], f32, kind="ExternalInput")
    out_d = nc.dram_tensor("out", [BPC, O, H // 2, W // 2], f32,
                           kind="ExternalOutput")

    with tile.TileContext(nc) as tc, ExitStack() as ctx:
        dram = ctx.enter_context(tc.tile_pool(name="dram", bufs=1,
                                              space="DRAM"))
        OFFd = dram.tile([BPC, 18, NN], bf16)
        VTd = dram.tile([BPC, 288, NN], bf16)       # c-major val rows
        PLd = dram.tile([BPC, O, NN // 4], bf16)    # pooled maxima
        cc_in = dram.tile([O, 2], f32)
        cc_out = dram.tile([O, 2], f32)

        consts = ctx.enter_context(tc.tile_pool(name="consts", bufs=1))
        wof_sb = consts.tile([C, 9 * 18], bf16)
        nc.sync.dma_start(
            wof_sb[:],
            bass.AP(woff_d[:].tensor, 0, [[18, C], [C * 18, 9], [1, 18]]))
        wd_sb = []
        for g in range(3):
            t = consts.tile([wd_d[g].shape[0], O], bf16, tag=f"wd{g}",
                            name=f"wd{g}")
            nc.sync.dma_start(t[:], wd_d[g][:])
            wd_sb.append(t)
        offb_sb = consts.tile([18, 1], f32)
        nc.sync.dma_start(offb_sb[:], offb_d[:])
        gam_sb = consts.tile([O, 1], f32)
        nc.sync.dma_start(gam_sb[:], gam_d[:])
        bet_sb = consts.tile([O, 1], f32)
        nc.sync.dma_start(bet_sb[:], bet_d[:])
        accp = consts.tile([O, 4 * NCH], f32)
        epsb = consts.tile([O, 1], f32)
        nc.vector.memset(epsb[:], EPS)
        sbias = []
        for s in range(3):
            t = consts.tile([128, 1], f32, tag=f"sb{s}", name=f"sb{s}")
            nc.vector.memset(t[:], float(-(s - 1)))
            sbias.append(t)

        # persistent padded image; edges zeroed once, interior rewritten
        xp_pool = ctx.enter_context(tc.tile_pool(name="xp", bufs=1))
        XT = xp_pool.tile([128, 5 * C * XH], bf16)   # 5 row-shifted copies
        nc.vector.memset(XT[:], 0.0)
        # c-major conv staging: 18 rows x 130 cols, 2 slots, edges zeroed
        xs_tiles = [xp_pool.tile([C, 18 * QW], bf16, tag=f"xs{i}",
                                 name=f"xs{i}") for i in range(2)]
        for t in xs_tiles:
            nc.vector.memset(t[:], 0.0)

        psum = ctx.enter_context(tc.tile_pool(name="psum", bufs=2,
                                              space="PSUM"))

        def v3(ap):
            return ap.rearrange("p (c x) -> p c x", x=W)

        for rep in range(reps):
            with tc.tile_pool(name="offp", bufs=1) as offp, \
                 tc.tile_pool(name="mskp", bufs=1) as mskp, \
                 tc.tile_pool(name="plp", bufs=5) as plp, \
                 tc.tile_pool(name="acp", bufs=2) as acp, \
                 tc.tile_pool(name="vcp", bufs=2) as vcp, \
                 tc.tile_pool(name="ocp", bufs=1) as ocp, \
                 tc.tile_pool(name="evp", bufs=2) as evp, \
                 tc.tile_pool(name="evq", bufs=1) as evq, \
                 tc.tile_pool(name="fin", bufs=1) as fin:

                def load_images(b):
                    """XT base copy from DRAM + 4 partition-shifted
                    SBUF->SBUF copies (big contiguous runs)."""
                    base = 2 * C * XH
                    xo = XT[:, base + 2:base + 2 + (C - 1) * XH + W]
                    xov = bass.AP(xo.tensor, xo.offset,
                                  [xo.ap[0], [XH, C], [1, W]])
                    src = x_d[b]
                    sv = bass.AP(src.tensor, src.offset,
                                 [[W, H], [H * W, C], [1, W]])
                    nc.sync.dma_start(xov, sv)
                    for d in (1, 3, 0, 4):
                        dy = d - 2
                        y0 = max(0, -dy)
                        ny = H - abs(dy)
                        dst = XT[y0:y0 + ny,
                                 d * C * XH:(d + 1) * C * XH]
                        srcv = XT[y0 + dy:y0 + dy + ny,
                                  base:base + C * XH]
                        eng = (nc.sync, nc.scalar)[d % 2]
                        eng.dma_start(dst, srcv)

                def offconv(b):
                    """3x3 conv -> OFFd[b]: PSUM chain of 9 shifted mms.

                    x rows [16ci-1, 16ci+17) staged per chunk into an
                    18-row x 130-col zero-edged c-major tile."""
                    for ci in range(NCH):
                        xs = xs_tiles[ci % 2]
                        r0 = 16 * ci - 1
                        rlo = max(0, r0)
                        rhi = min(H, r0 + 18)
                        if ci == 0:
                            nc.vector.memset(xs[:, 1:1 + W], 0.0)
                        if ci == NCH - 1:
                            nc.vector.memset(
                                xs[:, 17 * QW + 1:17 * QW + 1 + W], 0.0)
                        dsto = (rlo - r0) * QW + 1
                        dst = xs[:, dsto:dsto + (rhi - rlo - 1) * QW + W]
                        dv = bass.AP(dst.tensor, dst.offset,
                                     [dst.ap[0], [QW, rhi - rlo], [1, W]])
                        eng = (nc.sync, nc.scalar)[ci % 2]
                        eng.dma_start(dv, x_d[b, :, rlo:rhi])
                        pso = psum.tile([O, CH], f32, tag="ps", name="pso")
                        for k in range(9):
                            ki, kj = divmod(k, 3)
                            base = ki * QW + kj
                            for q4 in range(CH // 512):
                                sl = xs[:, base + q4 * 4 * QW:
                                        base + q4 * 4 * QW + 3 * QW + W]
                                rhs = bass.AP(sl.tensor, sl.offset,
                                              [sl.ap[0], [QW, 4], [1, W]])
                                nc.tensor.matmul(
                                    pso[0:18, q4 * 512:(q4 + 1) * 512],
                                    wof_sb[:, k * 18:(k + 1) * 18], rhs,
                                    start=(k == 0), stop=(k == 8))
                        oc = ocp.tile([18, CH], bf16, tag="oc", name="oc")
                        nc.scalar.activation(oc[:], pso[0:18, :],
                                             AF.Identity, bias=offb_sb[:])
                        nc.scalar.dma_start(
                            OFFd[b, :, ci * CH:(ci + 1) * CH], oc[:])

                def masks(b):
                    """offT -> vy/vx -> m81[y, (sy, k, sx, x)]."""
                    offT = offp.tile([128, 18 * W], bf16, tag="offT",
                                     name="offT")
                    src = OFFd[b]
                    nc.sync.dma_start(
                        offT[:],
                        bass.AP(src.tensor, src.offset,
                                [[W, 128], [NN, 18], [1, W]]))
                    vy = mskp.tile([128, 27 * W], bf16, tag="vy", name="vy")
                    vx = mskp.tile([128, 27 * W], bf16, tag="vx", name="vx")
                    tmp = mskp.tile([128, 9 * W], bf16, tag="tmp",
                                    name="tmp")
                    ov = offT[:]
                    for ax, vt in ((0, vy), (1, vx)):
                        dsl = bass.AP(ov.tensor, ov.offset + ax * W,
                                      [ov.ap[0], [2 * W, 9], [1, W]])
                        for s in range(3):
                            nc.scalar.activation(tmp[:], dsl, AF.Abs,
                                                 bias=sbias[s][:])
                            nc.scalar.activation(
                                vt[:, s * 9 * W:(s + 1) * 9 * W], tmp[:],
                                AF.Relu, bias=1.0, scale=-1.0)
                    m81 = mskp.tile([128, 81 * W], bf16, tag="m81",
                                    name="m81")
                    vyv = vy[:]
                    vxv = vx[:]
                    for sy in range(3):
                        # out [y, (9k, (3sx, x))] = vy[sy-block k] bcast sx
                        #                         * vx[(k, sx)]
                        mo = m81[:, sy * 27 * W:(sy + 1) * 27 * W]
                        mov = bass.AP(mo.tensor, mo.offset,
                                      [mo.ap[0], [3 * W, 9], [1, 3 * W]])
                        in0 = bass.AP(vyv.tensor,
                                      vyv.offset + sy * 9 * W,
                                      [vyv.ap[0], [W, 9], [0, 3], [1, W]])
                        in1 = bass.AP(vxv.tensor, vxv.offset,
                                      [vxv.ap[0], [W, 9], [9 * W, 3],
                                       [1, W]])
                        nc.vector.tensor_tensor(mov, in0, in1, A.mult)
                    return m81

                def emit_join(b, pend):
                    """vt = accA + accB for a finished k; scatter to VTd
                    in channel-major row order (big-run read side)."""
                    k, accA, accB = pend
                    vt = plp.tile([128, CW], bf16, tag="pt", name="vtj")
                    nc.vector.tensor_add(v3(vt[:]), v3(accA[:]),
                                         v3(accB[:]))
                    dd = VTd[b]
                    dst = bass.AP(dd.tensor, dd.offset + k * C * NN,
                                  [[W, H], [NN, C], [1, W]])
                    eng = (nc.sync, nc.scalar)[k % 2]
                    eng.dma_start(dst, vt[:])

                def deform_k(b, k, m81, pend):
                    """val_k[y, (c,x)] = sum_s m81-plane * XT-slice.

                    Products + 4 adds on DVE; planes 5-8 summed on gpsimd
                    off the critical path; the accA+accB join for the
                    PREVIOUS k is emitted mid-stream so DVE never waits
                    on gpsimd."""
                    ki, kj = divmod(k, 3)
                    planes = []
                    accA = acp.tile([128, CW], bf16, tag="accA",
                                    name="accA")
                    accB = acp.tile([128, CW], bf16, tag="accB",
                                    name="accB")
                    m81v = m81[:]
                    xtv = XT[:]
                    for si in range(9):
                        sy, sx = divmod(si, 3)
                        d = ki + sy           # 0..4 row-shift version
                        dx = kj + sx          # 0..4 col offset in xhat
                        moff = ((sy * 9 + k) * 3 + sx) * W
                        min1 = bass.AP(m81v.tensor, m81v.offset + moff,
                                       [m81v.ap[0], [0, C], [1, W]])
                        xin0 = bass.AP(xtv.tensor,
                                       xtv.offset + d * C * XH + dx,
                                       [xtv.ap[0], [XH, C], [1, W]])
                        pt = plp.tile([128, CW], bf16, tag="pt",
                                      name=f"pt{si}")
                        nc.vector.tensor_tensor(v3(pt[:]), xin0, min1,
                                                A.mult)
                        planes.append(pt)
                        if si == 1:
                            nc.vector.tensor_add(v3(accA[:]),
                                                 v3(planes[0][:]),
                                                 v3(planes[1][:]))
                        elif 2 <= si <= 4:
                            nc.vector.tensor_add(v3(accA[:]), v3(accA[:]),
                                                 v3(planes[si][:]))
                        elif si == 6:
                            nc.gpsimd.tensor_add(v3(accB[:]),
                                                 v3(planes[5][:]),
                                                 v3(planes[6][:]))
                        elif si >= 7:
                            nc.gpsimd.tensor_add(v3(accB[:]), v3(accB[:]),
                                                 v3(planes[si][:]))
                        if si == 2 and pend is not None:
                            emit_join(b, pend)
                    return (k, accA, accB)

                def final_chunk(b, ci, pooled_sl):
                    # gather c-major val tiles for this chunk from VTd
                    vals = []
                    for g, (kb, ng) in enumerate(KGROUPS):
                        vtile = vcp.tile([ng * C, CH], bf16, tag=f"val{g}",
                                         name=f"val{g}")
                        src = VTd[b]
                        inap = bass.AP(
                            src.tensor,
                            src.offset + kb * C * NN + ci * CH,
                            [[NN, ng * C], [1, CH]])
                        eng = (nc.sync, nc.scalar)[(ci + g) % 2]
                        eng.dma_start(vtile[:], inap)
                        vals.append(vtile)
                    ps = psum.tile([O, CH], f32, tag="ps", name="ps")
                    for g in range(3):
                        for q4 in range(CH // 512):
                            nc.tensor.matmul(
                                ps[:, q4 * 512:(q4 + 1) * 512],
                                wd_sb[g][:],
                                vals[g][:, q4 * 512:(q4 + 1) * 512],
                                start=(g == 0), stop=(g == 2))
                    col = 2 * (NCH * b + ci)
                    scr = evp.tile([O, CH], bf16, tag="scr", name="scr")
                    nc.scalar.activation(scr[:], ps[:], AF.Identity,
                                         accum_out=accp[:, col:col + 1])
                    rv = scr[:, :].rearrange("p (h w) -> p h w", w=W)
                    pw = evq.tile([O, CH // 2], bf16, tag="pw", name="pw")
                    pwv = pw[:, :].rearrange("p (h w) -> p h w", w=W // 2)
                    nc.vector.tensor_max(pwv, rv[:, :, 0:W:2],
                                         rv[:, :, 1:W:2])
                    pw3 = pw[:, :].rearrange("p (h w) -> p h w", w=W // 2)
                    mxs = evp.tile([O, CH // 4], bf16, tag="mxs",
                                   name="mxs")
                    nc.vector.tensor_max(
                        mxs[:].rearrange("p (h w) -> p h w", w=W // 2),
                        pw3[:, 0:16:2], pw3[:, 1:16:2])
                    nc.scalar.activation(scr[:], scr[:], AF.Square,
                                         accum_out=accp[:, col + 1:col + 2])
                    nc.sync.dma_start(pooled_sl, mxs[:])

                # ---------------- main schedule ----------------
                def deform_img(b, m81):
                    pend = None
                    for k in range(9):
                        pend = deform_k(b, k, m81, pend)
                    emit_join(b, pend)

                def finals_img(b):
                    for ci in range(NCH):
                        final_chunk(b, ci,
                                    PLd[b, :, ci * (CH // 4):
                                        (ci + 1) * (CH // 4)])

                load_images(0)
                offconv(0)
                m81_0 = masks(0)
                deform_img(0, m81_0)
                load_images(1)
                offconv(1)
                m81_1 = masks(1)
                finals_img(0)
                deform_img(1, m81_1)
                finals_img(1)

                # ---- BN: combine partials, allreduce across cores ----
                s12 = fin.tile([O, 2], f32, tag="s12", name="s12")
                nc.vector.tensor_add(accp[:, 0:16], accp[:, 0:16],
                                     accp[:, 16:32])
                nc.vector.tensor_add(accp[:, 0:8], accp[:, 0:8],
                                     accp[:, 8:16])
                nc.vector.tensor_add(accp[:, 0:4], accp[:, 0:4],
                                     accp[:, 4:8])
                nc.vector.tensor_add(s12[:, :], accp[:, 0:2], accp[:, 2:4])
                nc.sync.dma_start(cc_in[:], s12[:])
                nc.gpsimd.collective_compute(
                    "AllReduce", mybir.AluOpType.add,
                    replica_groups=[list(range(NCORES))],
                    ins=[cc_in.opt()], outs=[cc_out.opt()])

                s12r = fin.tile([O, 2], f32, tag="s12r", name="s12r")
                nc.sync.dma_start(s12r[:], cc_out[:])
                mr_ = fin.tile([O, 1], f32, tag="mr_", name="mr_")
                nc.vector.tensor_scalar_mul(mr_[:], s12r[:, 0:1],
                                            1.0 / NTOT)
                ex2 = fin.tile([O, 1], f32, tag="ex2", name="ex2")
                nc.vector.tensor_scalar_mul(ex2[:], s12r[:, 1:2],
                                            1.0 / NTOT)
                msq = fin.tile([O, 1], f32, tag="msq", name="msq")
                nc.vector.tensor_mul(msq[:], mr_[:], mr_[:])
                var = fin.tile([O, 1], f32, tag="var", name="var")
                nc.vector.tensor_sub(var[:], ex2[:], msq[:])
                sd = fin.tile([O, 1], f32, tag="sd", name="sd")
                nc.scalar.activation(sd[:], var[:], AF.Sqrt, bias=epsb[:])
                inv = fin.tile([O, 1], f32, tag="inv", name="inv")
                nc.vector.reciprocal(inv[:], sd[:])
                scf = fin.tile([O, 1], f32, tag="scf", name="scf")
                nc.vector.tensor_mul(scf[:], gam_sb[:], inv[:])
                tmp2 = fin.tile([O, 1], f32, tag="tmp2", name="tmp2")
                nc.vector.tensor_mul(tmp2[:], mr_[:], scf[:])
                bif = fin.tile([O, 1], f32, tag="bif", name="bif")
                nc.vector.tensor_sub(bif[:], bet_sb[:], tmp2[:])

                # ---- affine + relu on pooled maxima + store ----
                for b in range(BPC):
                    for q in range(16):
                        sl = PLd[b, :, q * 256:(q + 1) * 256]
                        plb = fin.tile([O, 256], bf16, tag="plb",
                                       name="plb")
                        nc.sync.dma_start(plb[:], sl)
                        r1 = fin.tile([O, 256], bf16, tag="r1", name="r1")
                        nc.vector.tensor_scalar(
                            r1[:], plb[:], scf[:], bif[:],
                            op0=mybir.AluOpType.mult,
                            op1=mybir.AluOpType.add)
                        po = fin.tile([O, 256], f32, tag="po", name="po")
                        nc.vector.tensor_scalar_max(po[:], r1[:], 0.0)
                        od = out_d[b]
                        nc.sync.dma_start(
                            bass.AP(od.tensor, od.offset + q * 256,
                                    [[NN // 4, O], [1, 256]]),
                            po[:, :])
    nc.compile()
    return nc


def _prep_inputs(x, offset_w, offset_b, conv_w, gamma, beta):
    """Host-side arrangement of weights into the layouts the kernel wants."""
    import ml_dtypes
    woff = np.zeros((9, C, 18), np.float32)
    for k in range(9):
        ki, kj = divmod(k, 3)
        woff[k] = offset_w[:, :, ki, kj].T
    wds = []
    for kb, ng in KGROUPS:
        blocks = []
        for kk in range(ng):
            ki, kj = divmod(kb + kk, 3)
            blocks.append(conv_w[:, :, ki, kj].T)      # [C, O]
        wds.append(np.ascontiguousarray(
            np.concatenate(blocks, axis=0)).astype(ml_dtypes.bfloat16))
    base = dict(
        woff=np.ascontiguousarray(woff).astype(ml_dtypes.bfloat16),
        wd0=wds[0], wd1=wds[1], wd2=wds[2],
        offb=offset_b.reshape(18, 1).astype(np.float32),
        gamma=gamma.reshape(O, 1).astype(np.float32),
        beta=beta.reshape(O, 1).astype(np.float32),
    )
    in_maps = []
    for ci in range(NCORES):
        m = dict(base)
        m["x_sh"] = np.ascontiguousarray(
            x[ci * BPC:(ci + 1) * BPC]).astype(ml_dtypes.bfloat16)
        in_maps.append(m)
    return in_maps


def _host_offsets(x, offset_w, offset_b):
    """offset = conv3x3(x, offset_w) + offset_b on host (|off|<1 check)."""
    xpad = np.pad(x, ((0, 0), (0, 0), (1, 1), (1, 1)))
    win = np.lib.stride_tricks.sliding_window_view(xpad, (3, 3), axis=(2, 3))
    cols = win.transpose(0, 2, 3, 1, 4, 5).reshape(B, NN, C * 9)
    w2 = offset_w.reshape(18, C * 9)
    off = cols @ w2.T.astype(np.float32)
    return off.reshape(B, H, W, 18).transpose(0, 3, 1, 2) + \
        offset_b.reshape(1, 18, 1, 1)


def _host_reference(x, offset_w, offset_b, conv_w, conv_b, gamma, beta):
    """Full numpy fallback (used only if some |offset| >= 1)."""
    off = _host_offsets(x, offset_w, offset_b).reshape(B, 9, 2, H, W)
    ki, kj = np.meshgrid(np.arange(3), np.arange(3), indexing="ij")
    base_y = (np.arange(H)[None, :, None] - 1 +
              ki.reshape(9)[:, None, None]).astype(np.float32)
    base_x = (np.arange(W)[None, None, :] - 1 +
              kj.reshape(9)[:, None, None]).astype(np.float32)
    py = off[:, :, 0] + base_y[None]
    px = off[:, :, 1] + base_x[None]
    y0 = np.floor(py).astype(np.int64)
    x0 = np.floor(px).astype(np.int64)
    wy = py - y0
    wx = px - x0
    bidx = np.arange(B)[:, None, None, None]

    def gather(iy, ix):
        valid = (iy >= 0) & (iy < H) & (ix >= 0) & (ix < W)
        v = x[bidx, :, np.clip(iy, 0, H - 1), np.clip(ix, 0, W - 1)]
        return np.where(valid[..., None], v, 0.0)

    val = (gather(y0, x0) * ((1 - wy) * (1 - wx))[..., None]
           + gather(y0, x0 + 1) * ((1 - wy) * wx)[..., None]
           + gather(y0 + 1, x0) * (wy * (1 - wx))[..., None]
           + gather(y0 + 1, x0 + 1) * (wy * wx)[..., None])
    out = np.einsum("bkhwc,ock->bohw", val, conv_w.reshape(O, C, 9),
                    optimize=True) + conv_b[None, :, None, None]
    m = out.mean(axis=(0, 2, 3), keepdims=True)
    v = out.var(axis=(0, 2, 3), keepdims=True)
    out = (out - m) / np.sqrt(v + EPS) * gamma[None, :, None, None] + \
        beta[None, :, None, None]
    out = np.maximum(out, 0.0)
    out = out.reshape(B, O, H // 2, 2, W // 2, 2).max(axis=(3, 5))
    return out.astype(np.float32)


def _get_nc(reps=1):
    key = ("nc", reps)
    if key not in _CACHE:
        _CACHE[key] = _build_nc(reps)
    return _CACHE[key]


def _run_device(in_maps, trace=False):
    from concourse import bass_utils
    nc = _get_nc()
    return bass_utils.run_bass_kernel_spmd(
        nc, in_maps, core_ids=list(range(NCORES)), trace=trace)


def kernel(x, offset_w, offset_b, mod_w, mod_b, conv_w, conv_b, gamma, beta,
           _trace=False, _return_results=False):
    x = np.asarray(x, np.float32)
    offset_w = np.asarray(offset_w, np.float32)
    offset_b = np.asarray(offset_b, np.float32)
    conv_w = np.asarray(conv_w, np.float32)
    conv_b = np.asarray(conv_b, np.float32)
    gamma = np.asarray(gamma, np.float32)
    beta = np.asarray(beta, np.float32)

    off = _host_offsets(x, offset_w, offset_b)
    if np.max(np.abs(off)) >= 0.999999 or np.min(gamma) < 0.0:
        return _host_reference(x, offset_w, offset_b, conv_w, conv_b,
                               gamma, beta)

    in_maps = _prep_inputs(x, offset_w, offset_b, conv_w, gamma, beta)
    res = _run_device(in_maps, trace=False)
    out = np.concatenate([res.results[i]["out"] for i in range(NCORES)],
                         axis=0)
    out = np.ascontiguousarray(out).astype(np.float32)
    if _return_results:
        return out, res
    return out


# revision 12
# speedup vs baseline: 1.4045x; 1.1396x over previous
"""Trainium2 Bass kernel for nn_DConv2dBlock (deformable conv block).

Pixel-major formulation (batch sharded 2 images per core across 8 cores):
  1. offset = 3x3 conv(x): PE PSUM chain of 9 shifted matmuls per chunk
     (rhs = shifted views of a zero-padded c-major image, no staging DMA).
  2. offsets permuted to pixel-major [y, (plane, x)]; triangle masks
     Lambda(dy - s) = relu(1 - |dy - s|) built by ACT; the 81 (sy, k, sx)
     mask planes m81[y, (sy,k,sx,x)] = vy * vx via 3 DVE ops per image.
  3. products in pixel-major [y, (c, x)]: for each (k,s) one DVE op
       p = m81-plane (broadcast over c via stride-0 AP) * XT-slice
     where XT[y, (dy+2, c, xhat)] holds 5 row-shifted copies of the
     x-padded image, so both shift axes are free-dim offsets and no mask
     fan-out DMA exists at all (the channel broadcast happens inside the
     DVE operand read).
  4. per (k, img): val_k = sum of 9 products; 5 adds on DVE, 3 on gpsimd.
  5. val_k dumped to DRAM (contiguous); re-read per chunk with a
     (c, y, x) gather into channel-major [(k,c), CH] tiles; PE contracts
     all 288 (k,c) rows in a 3-matmul PSUM chain per chunk.
  6. BN stats via ACT accum_out on PSUM evacuation; 2x2 maxpool inline on
     pre-BN activations (commutes with the affine since scf >= 0); 8-core
     AllReduce of (S1, S2); tiny affine+relu on pooled maxima.

The modulator branch of the reference is dead code and skipped.
conv bias cancels inside BatchNorm and is skipped.
Requires max|offset| < 1 (checked on host; falls back to a full host
computation in the measure-zero case where it does not hold).
"""

import os
import sys
import numpy as np

for _p in ("/opt/trn_rl_repo",):
    if os.path.isdir(_p) and _p not in sys.path:
        sys.path.insert(0, _p)

B, C, H, W = 16, 32, 128, 128
O = 64
NCORES = 8
BPC = B // NCORES          # images per core
NN = H * W                 # pixels per image (16384)
EPS = 1e-5
NTOT = float(B * NN)
CH = 2048                  # chunk: 16 image rows
NCH = NN // CH             # chunks per image (8)
XH = W + 4                 # padded row width for XT (132)
QW = W + 2                 # padded cols in c-major image (130)
CW = C * W                 # free size of a (c, x) plane (4096)
KGROUPS = [(0, 4), (4, 4), (8, 1)]

_CACHE = {}
_UPTO = "full"   # "deform" | "finals" | "coll" | "full"


def _build_nc(reps=1):
    import concourse.bass as bass
    import concourse.bacc as bacc
    import concourse.mybir as mybir
    from concourse import tile
    from contextlib import ExitStack

    f32 = mybir.dt.float32
    bf16 = mybir.dt.bfloat16
    AF = mybir.ActivationFunctionType
    A = mybir.AluOpType

    nc = bacc.Bacc(num_devices=NCORES)
    x_d = nc.dram_tensor("x_sh", [BPC, C, H, W], bf16, kind="ExternalInput")
    woff_d = nc.dram_tensor("woff", [9, C, 18], bf16, kind="ExternalInput")
    wd_d = [
        nc.dram_tensor("wd0", [128, O], bf16, kind="ExternalInput"),
        nc.dram_tensor("wd1", [128, O], bf16, kind="ExternalInput"),
        nc.dram_tensor("wd2", [32, O], bf16, kind="ExternalInput"),
    ]
    offb_d = nc.dram_tensor("offb", [18, 1], f32, kind="ExternalInput")
    gam_d = nc.dram_tensor("gamma", [O, 1], f32, kind="ExternalInput")
    bet_d = nc.dram_tensor("beta", [O, 1], f32, kind="ExternalInput")
    out_d = nc.dram_tensor("out", [BPC, O, H // 2, W // 2], f32,
                           kind="ExternalOutput")

    with tile.TileContext(nc) as tc, ExitStack() as ctx:
        dram = ctx.enter_context(tc.tile_pool(name="dram", bufs=1,
                                              space="DRAM"))
        OFFd = dram.tile([BPC, 18, NN], bf16)
        VTd = dram.tile([BPC, 288, NN], bf16)       # c-major val rows
        PLd = dram.tile([BPC, O, NN // 4], bf16)    # pooled maxima
        cc_in = dram.tile([O, 2], f32)
        cc_out = dram.tile([O, 2], f32)

        consts = ctx.enter_context(tc.tile_pool(name="consts", bufs=1))
        wof_sb = consts.tile([C, 9 * 18], bf16)
        nc.sync.dma_start(
            wof_sb[:],
            bass.AP(woff_d[:].tensor, 0, [[18, C], [C * 18, 9], [1, 18]]))
        wd_sb = []
        for g in range(3):
            t = consts.tile([wd_d[g].shape[0], O], bf16, tag=f"wd{g}",
                            name=f"wd{g}")
            nc.sync.dma_start(t[:], wd_d[g][:])
            wd_sb.append(t)
        offb_sb = consts.tile([18, 1], f32)
        nc.sync.dma_start(offb_sb[:], offb_d[:])
        gam_sb = consts.tile([O, 1], f32)
        nc.sync.dma_start(gam_sb[:], gam_d[:])
        bet_sb = consts.tile([O, 1], f32)
        nc.sync.dma_start(bet_sb[:], bet_d[:])
        accp = consts.tile([O, 4 * NCH], f32)
        epsb = consts.tile([O, 1], f32)
        nc.vector.memset(epsb[:], EPS)
        sbias = []
        for s in range(3):
            t = consts.tile([128, 1], f32, tag=f"sb{s}", name=f"sb{s}")
            nc.vector.memset(t[:], float(-(s - 1)))
            sbias.append(t)

        # persistent padded image; edges zeroed once, interior rewritten
        xp_pool = ctx.enter_context(tc.tile_pool(name="xp", bufs=1))
        XT = xp_pool.tile([128, 5 * C * XH], bf16)   # 5 row-shifted copies
        nc.vector.memset(XT[:], 0.0)
        # c-major conv staging: 18 rows x 130 cols, 2 slots, edges zeroed
        xs_tiles = [xp_pool.tile([C, 18 * QW], bf16, tag=f"xs{i}",
                                 name=f"xs{i}") for i in range(2)]
        for t in xs_tiles:
            nc.vector.memset(t[:], 0.0)

        psum = ctx.enter_context(tc.tile_pool(name="psum", bufs=2,
                                              space="PSUM"))

        def v3(ap):
            return ap.rearrange("p (c x) -> p c x", x=W)

        for rep in range(reps):
            with tc.tile_pool(name="offp", bufs=1) as offp, \
                 tc.tile_pool(name="mskp", bufs=1) as mskp, \
                 tc.tile_pool(name="plp", bufs=5) as plp, \
                 tc.tile_pool(name="acp", bufs=2) as acp, \
                 tc.tile_pool(name="vcp", bufs=2) as vcp, \
                 tc.tile_pool(name="ocp", bufs=1) as ocp, \
                 tc.tile_pool(name="evp", bufs=2) as evp, \
                 tc.tile_pool(name="evq", bufs=1) as evq, \
                 tc.tile_pool(name="fin", bufs=1) as fin:

                def load_images(b):
                    """XT base copy from DRAM + 4 partition-shifted
                    SBUF->SBUF copies (big contiguous runs)."""
                    base = 2 * C * XH
                    xo = XT[:, base + 2:base + 2 + (C - 1) * XH + W]
                    xov = bass.AP(xo.tensor, xo.offset,
                                  [xo.ap[0], [XH, C], [1, W]])
                    src = x_d[b]
                    sv = bass.AP(src.tensor, src.offset,
                                 [[W, H], [H * W, C], [1, W]])
                    nc.sync.dma_start(xov, sv)
                    for d in (1, 3, 0, 4):
                        dy = d - 2
                        y0 = max(0, -dy)
                        ny = H - abs(dy)
                        dst = XT[y0:y0 + ny,
                                 d * C * XH:(d + 1) * C * XH]
                        srcv = XT[y0 + dy:y0 + dy + ny,
                                  base:base + C * XH]
                        eng = (nc.sync, nc.scalar)[d % 2]
                        eng.dma_start(dst, srcv)

                def offconv(b):
                    """3x3 conv -> OFFd[b]: PSUM chain of 9 shifted mms.

                    x rows [16ci-1, 16ci+17) staged per chunk into an
                    18-row x 130-col zero-edged c-major tile."""
                    for ci in range(NCH):
                        xs = xs_tiles[ci % 2]
                        r0 = 16 * ci - 1
                        rlo = max(0, r0)
                        rhi = min(H, r0 + 18)
                        if ci == 0:
                            nc.vector.memset(xs[:, 1:1 + W], 0.0)
                        if ci == NCH - 1:
                            nc.vector.memset(
                                xs[:, 17 * QW + 1:17 * QW + 1 + W], 0.0)
                        dsto = (rlo - r0) * QW + 1
                        dst = xs[:, dsto:dsto + (rhi - rlo - 1) * QW + W]
                        dv = bass.AP(dst.tensor, dst.offset,
                                     [dst.ap[0], [QW, rhi - rlo], [1, W]])
                        eng = (nc.sync, nc.scalar)[ci % 2]
                        eng.dma_start(dv, x_d[b, :, rlo:rhi])
                        pso = psum.tile([O, CH], f32, tag="ps", name="pso")
                        for k in range(9):
                            ki, kj = divmod(k, 3)
                            base = ki * QW + kj
                            for q4 in range(CH // 512):
                                sl = xs[:, base + q4 * 4 * QW:
                                        base + q4 * 4 * QW + 3 * QW + W]
                                rhs = bass.AP(sl.tensor, sl.offset,
                                              [sl.ap[0], [QW, 4], [1, W]])
                                nc.tensor.matmul(
                                    pso[0:18, q4 * 512:(q4 + 1) * 512],
                                    wof_sb[:, k * 18:(k + 1) * 18], rhs,
                                    start=(k == 0), stop=(k == 8))
                        oc = ocp.tile([18, CH], bf16, tag="oc", name="oc")
                        nc.scalar.activation(oc[:], pso[0:18, :],
                                             AF.Identity, bias=offb_sb[:])
                        nc.scalar.dma_start(
                            OFFd[b, :, ci * CH:(ci + 1) * CH], oc[:])

                def masks(b):
                    """offT -> vy/vx -> m81[y, (sy, k, sx, x)]."""
                    offT = offp.tile([128, 18 * W], bf16, tag="offT",
                                     name="offT")
                    src = OFFd[b]
                    nc.sync.dma_start(
                        offT[:],
                        bass.AP(src.tensor, src.offset,
                                [[W, 128], [NN, 18], [1, W]]))
                    vy = mskp.tile([128, 27 * W], bf16, tag="vy", name="vy")
                    vx = mskp.tile([128, 27 * W], bf16, tag="vx", name="vx")
                    ov = offT[:]
                    for ax, vt in ((0, vy), (1, vx)):
                        dsl = bass.AP(ov.tensor, ov.offset + ax * W,
                                      [ov.ap[0], [2 * W, 9], [1, W]])
                        for s in range(3):
                            sl = vt[:, s * 9 * W:(s + 1) * 9 * W]
                            nc.scalar.activation(sl, dsl, AF.Abs,
                                                 bias=sbias[s][:])
                            nc.scalar.activation(sl, sl, AF.Relu,
                                                 bias=1.0, scale=-1.0)
                    m81 = mskp.tile([128, 81 * W], bf16, tag="m81",
                                    name="m81")
                    vyv = vy[:]
                    vxv = vx[:]
                    for sy in range(3):
                        # out [y, (9k, (3sx, x))] = vy[sy-block k] bcast sx
                        #                         * vx[(k, sx)]
                        mo = m81[:, sy * 27 * W:(sy + 1) * 27 * W]
                        mov = bass.AP(mo.tensor, mo.offset,
                                      [mo.ap[0], [3 * W, 9], [1, 3 * W]])
                        in0 = bass.AP(vyv.tensor,
                                      vyv.offset + sy * 9 * W,
                                      [vyv.ap[0], [W, 9], [0, 3], [1, W]])
                        in1 = bass.AP(vxv.tensor, vxv.offset,
                                      [vxv.ap[0], [W, 9], [9 * W, 3],
                                       [1, W]])
                        nc.vector.tensor_tensor(mov, in0, in1, A.mult)
                    return m81

                def deform_k(b, k, m81):
                    """val_k[y, (c,x)] = sum_s m81-plane * XT-slice.

                    gpsimd sums the EARLY planes (p0..p2, available while
                    DVE is still producing) and does the final join, which
                    only the leg1 DMA consumes - so the slow gpsimd chain
                    never blocks DVE."""
                    ki, kj = divmod(k, 3)
                    planes = []
                    accA = acp.tile([128, CW], bf16, tag="accA",
                                    name="accA")
                    accB = acp.tile([128, CW], bf16, tag="accB",
                                    name="accB")
                    m81v = m81[:]
                    xtv = XT[:]
                    for si in range(9):
                        sy, sx = divmod(si, 3)
                        d = ki + sy           # 0..4 row-shift version
                        dx = kj + sx          # 0..4 col offset in xhat
                        moff = ((sy * 9 + k) * 3 + sx) * W
                        min1 = bass.AP(m81v.tensor, m81v.offset + moff,
                                       [m81v.ap[0], [0, C], [1, W]])
                        xin0 = bass.AP(xtv.tensor,
                                       xtv.offset + d * C * XH + dx,
                                       [xtv.ap[0], [XH, C], [1, W]])
                        pt = plp.tile([128, CW], bf16, tag="pt",
                                      name=f"pt{si}")
                        nc.vector.tensor_tensor(v3(pt[:]), xin0, min1,
                                                A.mult)
                        planes.append(pt)
                        if si == 1:
                            nc.gpsimd.tensor_add(v3(accB[:]),
                                                 v3(planes[0][:]),
                                                 v3(planes[1][:]))
                        elif si == 2:
                            nc.gpsimd.tensor_add(v3(accB[:]), v3(accB[:]),
                                                 v3(planes[2][:]))
                        elif si == 4:
                            nc.vector.tensor_add(v3(accA[:]),
                                                 v3(planes[3][:]),
                                                 v3(planes[4][:]))
                        elif si >= 5:
                            nc.vector.tensor_add(v3(accA[:]), v3(accA[:]),
                                                 v3(planes[si][:]))
                    vt = plp.tile([128, CW], bf16, tag="pt", name="vtj")
                    nc.gpsimd.tensor_add(v3(vt[:]), v3(accA[:]),
                                         v3(accB[:]))
                    dd = VTd[b]
                    dst = bass.AP(dd.tensor, dd.offset + k * C * NN,
                                  [[W, H], [NN, C], [1, W]])
                    eng = (nc.sync, nc.scalar)[k % 2]
                    eng.dma_start(dst, vt[:])

                def final_chunk(b, ci, pooled_sl):
                    # gather c-major val tiles for this chunk from VTd
                    vals = []
                    for g, (kb, ng) in enumerate(KGROUPS):
                        vtile = vcp.tile([ng * C, CH], bf16, tag=f"val{g}",
                                         name=f"val{g}")
                        src = VTd[b]
                        inap = bass.AP(
                            src.tensor,
                            src.offset + kb * C * NN + ci * CH,
                            [[NN, ng * C], [1, CH]])
                        eng = (nc.sync, nc.scalar)[(ci + g) % 2]
                        eng.dma_start(vtile[:], inap)
                        vals.append(vtile)
                    ps = psum.tile([O, CH], f32, tag="ps", name="ps")
                    for g in range(3):
                        for q4 in range(CH // 512):
                            nc.tensor.matmul(
                                ps[:, q4 * 512:(q4 + 1) * 512],
                                wd_sb[g][:],
                                vals[g][:, q4 * 512:(q4 + 1) * 512],
                                start=(g == 0), stop=(g == 2))
                    col = 2 * (NCH * b + ci)
                    scr = evp.tile([O, CH], bf16, tag="scr", name="scr")
                    nc.scalar.activation(scr[:], ps[:], AF.Identity,
                                         accum_out=accp[:, col:col + 1])
                    rv = scr[:, :].rearrange("p (h w) -> p h w", w=W)
                    pw = evq.tile([O, CH // 2], bf16, tag="pw", name="pw")
                    pwv = pw[:, :].rearrange("p (h w) -> p h w", w=W // 2)
                    nc.vector.tensor_max(pwv, rv[:, :, 0:W:2],
                                         rv[:, :, 1:W:2])
                    pw3 = pw[:, :].rearrange("p (h w) -> p h w", w=W // 2)
                    mxs = evp.tile([O, CH // 4], bf16, tag="mxs",
                                   name="mxs")
                    nc.vector.tensor_max(
                        mxs[:].rearrange("p (h w) -> p h w", w=W // 2),
                        pw3[:, 0:16:2], pw3[:, 1:16:2])
                    nc.scalar.activation(scr[:], scr[:], AF.Square,
                                         accum_out=accp[:, col + 1:col + 2])
                    nc.sync.dma_start(pooled_sl, mxs[:])

                # ---------------- main schedule ----------------
                def deform_img(b, m81):
                    for k in range(9):
                        deform_k(b, k, m81)

                def finals_img(b):
                    for ci in range(NCH):
                        final_chunk(b, ci,
                                    PLd[b, :, ci * (CH // 4):
                                        (ci + 1) * (CH // 4)])

                if _UPTO != "coll":
                    load_images(0)
                    offconv(0)
                    m81_0 = masks(0)
                    deform_img(0, m81_0)
                    load_images(1)
                    offconv(1)
                    m81_1 = masks(1)
                    if _UPTO != "deform":
                        finals_img(0)
                    deform_img(1, m81_1)
                    if _UPTO != "deform":
                        finals_img(1)
                if _UPTO in ("deform", "finals"):
                    # keep the tail structure alive without the collective
                    nc.vector.memset(accp[:, 0:32], 1.0)

                # ---- BN: combine partials, allreduce across cores ----
                if _UPTO == "coll":
                    nc.vector.memset(accp[:, 0:32], 1.0)
                s12 = fin.tile([O, 2], f32, tag="s12", name="s12")
                nc.vector.tensor_add(accp[:, 0:16], accp[:, 0:16],
                                     accp[:, 16:32])
                nc.vector.tensor_add(accp[:, 0:8], accp[:, 0:8],
                                     accp[:, 8:16])
                nc.vector.tensor_add(accp[:, 0:4], accp[:, 0:4],
                                     accp[:, 4:8])
                nc.vector.tensor_add(s12[:, :], accp[:, 0:2], accp[:, 2:4])
                nc.sync.dma_start(cc_in[:], s12[:])
                nc.gpsimd.collective_compute(
                    "AllReduce", mybir.AluOpType.add,
                    replica_groups=[list(range(NCORES))],
                    ins=[cc_in.opt()], outs=[cc_out.opt()])

                s12r = fin.tile([O, 2], f32, tag="s12r", name="s12r")
                nc.sync.dma_start(s12r[:], cc_out[:])
                mr_ = fin.tile([O, 1], f32, tag="mr_", name="mr_")
                nc.vector.tensor_scalar_mul(mr_[:], s12r[:, 0:1],
                                            1.0 / NTOT)
                ex2 = fin.tile([O, 1], f32, tag="ex2", name="ex2")
                nc.vector.tensor_scalar_mul(ex2[:], s12r[:, 1:2],
                                            1.0 / NTOT)
                msq = fin.tile([O, 1], f32, tag="msq", name="msq")
                nc.vector.tensor_mul(msq[:], mr_[:], mr_[:])
                var = fin.tile([O, 1], f32, tag="var", name="var")
                nc.vector.tensor_sub(var[:], ex2[:], msq[:])
                sd = fin.tile([O, 1], f32, tag="sd", name="sd")
                nc.scalar.activation(sd[:], var[:], AF.Sqrt, bias=epsb[:])
                inv = fin.tile([O, 1], f32, tag="inv", name="inv")
                nc.vector.reciprocal(inv[:], sd[:])
                scf = fin.tile([O, 1], f32, tag="scf", name="scf")
                nc.vector.tensor_mul(scf[:], gam_sb[:], inv[:])
                tmp2 = fin.tile([O, 1], f32, tag="tmp2", name="tmp2")
                nc.vector.tensor_mul(tmp2[:], mr_[:], scf[:])
                bif = fin.tile([O, 1], f32, tag="bif", name="bif")
                nc.vector.tensor_sub(bif[:], bet_sb[:], tmp2[:])

                # ---- affine + relu on pooled maxima + store ----
                for b in range(BPC):
                    for q in range(16):
                        sl = PLd[b, :, q * 256:(q + 1) * 256]
                        plb = fin.tile([O, 256], bf16, tag="plb",
                                       name="plb")
                        nc.sync.dma_start(plb[:], sl)
                        r1 = fin.tile([O, 256], bf16, tag="r1", name="r1")
                        nc.vector.tensor_scalar(
                            r1[:], plb[:], scf[:], bif[:],
                            op0=mybir.AluOpType.mult,
                            op1=mybir.AluOpType.add)
                        po = fin.tile([O, 256], f32, tag="po", name="po")
                        nc.vector.tensor_scalar_max(po[:], r1[:], 0.0)
                        od = out_d[b]
                        nc.sync.dma_start(
                            bass.AP(od.tensor, od.offset + q * 256,
                                    [[NN // 4, O], [1, 256]]),
                            po[:, :])
    nc.compile()
    return nc


def _prep_inputs(x, offset_w, offset_b, conv_w, gamma, beta):
    """Host-side arrangement of weights into the layouts the kernel wants."""
    import ml_dtypes
    woff = np.zeros((9, C, 18), np.float32)
    for k in range(9):
        ki, kj = divmod(k, 3)
        woff[k] = offset_w[:, :, ki, kj].T
    wds = []
    for kb, ng in KGROUPS:
        blocks = []
        for kk in range(ng):
            ki, kj = divmod(kb + kk, 3)
            blocks.append(conv_w[:, :, ki, kj].T)      # [C, O]
        wds.append(np.ascontiguousarray(
            np.concatenate(blocks, axis=0)).astype(ml_dtypes.bfloat16))
    base = dict(
        woff=np.ascontiguousarray(woff).astype(ml_dtypes.bfloat16),
        wd0=wds[0], wd1=wds[1], wd2=wds[2],
        offb=offset_b.reshape(18, 1).astype(np.float32),
        gamma=gamma.reshape(O, 1).astype(np.float32),
        beta=beta.reshape(O, 1).astype(np.float32),
    )
    in_maps = []
    for ci in range(NCORES):
        m = dict(base)
        m["x_sh"] = np.ascontiguousarray(
            x[ci * BPC:(ci + 1) * BPC]).astype(ml_dtypes.bfloat16)
        in_maps.append(m)
    return in_maps


def _host_offsets(x, offset_w, offset_b):
    """offset = conv3x3(x, offset_w) + offset_b on host (|off|<1 check)."""
    xpad = np.pad(x, ((0, 0), (0, 0), (1, 1), (1, 1)))
    win = np.lib.stride_tricks.sliding_window_view(xpad, (3, 3), axis=(2, 3))
    cols = win.transpose(0, 2, 3, 1, 4, 5).reshape(B, NN, C * 9)
    w2 = offset_w.reshape(18, C * 9)
    off = cols @ w2.T.astype(np.float32)
    return off.reshape(B, H, W, 18).transpose(0, 3, 1, 2) + \
        offset_b.reshape(1, 18, 1, 1)


def _host_reference(x, offset_w, offset_b, conv_w, conv_b, gamma, beta):
    """Full numpy fallback (used only if some |offset| >= 1)."""
    off = _host_offsets(x, offset_w, offset_b).reshape(B, 9, 2, H, W)
    ki, kj = np.meshgrid(np.arange(3), np.arange(3), indexing="ij")
    base_y = (np.arange(H)[None, :, None] - 1 +
              ki.reshape(9)[:, None, None]).astype(np.float32)
    base_x = (np.arange(W)[None, None, :] - 1 +
              kj.reshape(9)[:, None, None]).astype(np.float32)
    py = off[:, :, 0] + base_y[None]
    px = off[:, :, 1] + base_x[None]
    y0 = np.floor(py).astype(np.int64)
    x0 = np.floor(px).astype(np.int64)
    wy = py - y0
    wx = px - x0
    bidx = np.arange(B)[:, None, None, None]

    def gather(iy, ix):
        valid = (iy >= 0) & (iy < H) & (ix >= 0) & (ix < W)
        v = x[bidx, :, np.clip(iy, 0, H - 1), np.clip(ix, 0, W - 1)]
        return np.where(valid[..., None], v, 0.0)

    val = (gather(y0, x0) * ((1 - wy) * (1 - wx))[..., None]
           + gather(y0, x0 + 1) * ((1 - wy) * wx)[..., None]
           + gather(y0 + 1, x0) * (wy * (1 - wx))[..., None]
           + gather(y0 + 1, x0 + 1) * (wy * wx)[..., None])
    out = np.einsum("bkhwc,ock->bohw", val, conv_w.reshape(O, C, 9),
                    optimize=True) + conv_b[None, :, None, None]
    m = out.mean(axis=(0, 2, 3), keepdims=True)
    v = out.var(axis=(0, 2, 3), keepdims=True)
    out = (out - m) / np.sqrt(v + EPS) * gamma[None, :, None, None] + \
        beta[None, :, None, None]
    out = np.maximum(out, 0.0)
    out = out.reshape(B, O, H // 2, 2, W // 2, 2).max(axis=(3, 5))
    return out.astype(np.float32)


def _get_nc(reps=1):
    key = ("nc", reps)
    if key not in _CACHE:
        _CACHE[key] = _build_nc(reps)
    return _CACHE[key]


def _run_device(in_maps, trace=False):
    from concourse import bass_utils
    nc = _get_nc()
    return bass_utils.run_bass_kernel_spmd(
        nc, in_maps, core_ids=list(range(NCORES)), trace=trace)


def kernel(x, offset_w, offset_b, mod_w, mod_b, conv_w, conv_b, gamma, beta,
           _trace=False, _return_results=False):
    x = np.asarray(x, np.float32)
    offset_w = np.asarray(offset_w, np.float32)
    offset_b = np.asarray(offset_b, np.float32)
    conv_w = np.asarray(conv_w, np.float32)
    conv_b = np.asarray(conv_b, np.float32)
    gamma = np.asarray(gamma, np.float32)
    beta = np.asarray(beta, np.float32)

    off = _host_offsets(x, offset_w, offset_b)
    if np.max(np.abs(off)) >= 0.999999 or np.min(gamma) < 0.0:
        return _host_reference(x, offset_w, offset_b, conv_w, conv_b,
                               gamma, beta)

    in_maps = _prep_inputs(x, offset_w, offset_b, conv_w, gamma, beta)
    res = _run_device(in_maps, trace=False)
    out = np.concatenate([res.results[i]["out"] for i in range(NCORES)],
                         axis=0)
    out = np.ascontiguousarray(out).astype(np.float32)
    if _return_results:
        return out, res
    return out


# revision 14
# speedup vs baseline: 1.4712x; 1.0475x over previous
"""Trainium2 Bass kernel for nn_DConv2dBlock (deformable conv block).

Pixel-major formulation (batch sharded 2 images per core across 8 cores):
  1. offset = 3x3 conv(x): PE PSUM chain of 9 shifted matmuls per chunk
     (rhs = shifted views of a zero-padded c-major image, no staging DMA).
  2. offsets permuted to pixel-major [y, (plane, x)]; triangle masks
     Lambda(dy - s) = relu(1 - |dy - s|) built by ACT; the 81 (sy, k, sx)
     mask planes m81[y, (sy,k,sx,x)] = vy * vx via 3 DVE ops per image.
  3. products in pixel-major [y, (c, x)]: for each (k,s) one DVE op
       p = m81-plane (broadcast over c via stride-0 AP) * XT-slice
     where XT[y, (dy+2, c, xhat)] holds 5 row-shifted copies of the
     x-padded image, so both shift axes are free-dim offsets and no mask
     fan-out DMA exists at all (the channel broadcast happens inside the
     DVE operand read).
  4. per (k, img): val_k = sum of 9 products; 5 adds on DVE, 3 on gpsimd.
  5. val_k dumped to DRAM (contiguous); re-read per chunk with a
     (c, y, x) gather into channel-major [(k,c), CH] tiles; PE contracts
     all 288 (k,c) rows in a 3-matmul PSUM chain per chunk.
  6. BN stats via ACT accum_out on PSUM evacuation; 2x2 maxpool inline on
     pre-BN activations (commutes with the affine since scf >= 0); 8-core
     AllReduce of (S1, S2); tiny affine+relu on pooled maxima.

The modulator branch of the reference is dead code and skipped.
conv bias cancels inside BatchNorm and is skipped.
Requires max|offset| < 1 (checked on host; falls back to a full host
computation in the measure-zero case where it does not hold).
"""

import os
import sys
import numpy as np

for _p in ("/opt/trn_rl_repo",):
    if os.path.isdir(_p) and _p not in sys.path:
        sys.path.insert(0, _p)

B, C, H, W = 16, 32, 128, 128
O = 64
NCORES = 8
BPC = B // NCORES          # images per core
NN = H * W                 # pixels per image (16384)
EPS = 1e-5
NTOT = float(B * NN)
CH = 2048                  # chunk: 16 image rows
NCH = NN // CH             # chunks per image (8)
XH = W + 4                 # padded row width for XT (132)
QW = W + 2                 # padded cols in c-major image (130)
CW = C * W                 # free size of a (c, x) plane (4096)
KGROUPS = [(0, 4), (4, 4), (8, 1)]

_CACHE = {}
_UPTO = "full"   # "deform" | "finals" | "coll" | "full"


def _build_nc(reps=1):
    import concourse.bass as bass
    import concourse.bacc as bacc
    import concourse.mybir as mybir
    from concourse import tile
    from contextlib import ExitStack

    f32 = mybir.dt.float32
    bf16 = mybir.dt.bfloat16
    AF = mybir.ActivationFunctionType
    A = mybir.AluOpType

    nc = bacc.Bacc(num_devices=NCORES)
    x_d = nc.dram_tensor("x_sh", [BPC, C, H, W], bf16, kind="ExternalInput")
    woff_d = nc.dram_tensor("woff", [9, C, 18], bf16, kind="ExternalInput")
    wd_d = [
        nc.dram_tensor("wd0", [128, O], bf16, kind="ExternalInput"),
        nc.dram_tensor("wd1", [128, O], bf16, kind="ExternalInput"),
        nc.dram_tensor("wd2", [32, O], bf16, kind="ExternalInput"),
    ]
    offb_d = nc.dram_tensor("offb", [18, 1], f32, kind="ExternalInput")
    gam_d = nc.dram_tensor("gamma", [O, 1], f32, kind="ExternalInput")
    bet_d = nc.dram_tensor("beta", [O, 1], f32, kind="ExternalInput")
    out_d = nc.dram_tensor("out", [BPC, O, H // 2, W // 2], f32,
                           kind="ExternalOutput")

    with tile.TileContext(nc) as tc, ExitStack() as ctx:
        dram = ctx.enter_context(tc.tile_pool(name="dram", bufs=1,
                                              space="DRAM"))
        OFFd = dram.tile([BPC, 18, NN], bf16)
        VTd = dram.tile([BPC, 288, NN], bf16)       # c-major val rows
        PLd = dram.tile([BPC, O, NN // 4], bf16)    # pooled maxima
        cc_in = dram.tile([O, 2], f32)
        cc_out = dram.tile([O, 2], f32)

        consts = ctx.enter_context(tc.tile_pool(name="consts", bufs=1))
        wof_sb = consts.tile([C, 9 * 18], bf16)
        nc.sync.dma_start(
            wof_sb[:],
            bass.AP(woff_d[:].tensor, 0, [[18, C], [C * 18, 9], [1, 18]]))
        wd_sb = []
        for g in range(3):
            t = consts.tile([wd_d[g].shape[0], O], bf16, tag=f"wd{g}",
                            name=f"wd{g}")
            nc.sync.dma_start(t[:], wd_d[g][:])
            wd_sb.append(t)
        offb_sb = consts.tile([18, 1], f32)
        nc.sync.dma_start(offb_sb[:], offb_d[:])
        gam_sb = consts.tile([O, 1], f32)
        nc.sync.dma_start(gam_sb[:], gam_d[:])
        bet_sb = consts.tile([O, 1], f32)
        nc.sync.dma_start(bet_sb[:], bet_d[:])
        accp = consts.tile([O, 4 * NCH], f32)
        epsb = consts.tile([O, 1], f32)
        nc.vector.memset(epsb[:], EPS)
        sbias = []
        for s in range(3):
            t = consts.tile([128, 1], f32, tag=f"sb{s}", name=f"sb{s}")
            nc.vector.memset(t[:], float(-(s - 1)))
            sbias.append(t)

        # persistent padded image; edges zeroed once, interior rewritten
        xp_pool = ctx.enter_context(tc.tile_pool(name="xp", bufs=1))
        XT = xp_pool.tile([128, 5 * C * XH], bf16)   # 5 row-shifted copies
        nc.vector.memset(XT[:], 0.0)
        # c-major conv staging: 18 rows x 130 cols, 2 slots, edges zeroed
        xs_tiles = [xp_pool.tile([C, 18 * QW], bf16, tag=f"xs{i}",
                                 name=f"xs{i}") for i in range(2)]
        for t in xs_tiles:
            nc.vector.memset(t[:], 0.0)

        psum = ctx.enter_context(tc.tile_pool(name="psum", bufs=2,
                                              space="PSUM"))

        def v3(ap):
            return ap.rearrange("p (c x) -> p c x", x=W)

        for rep in range(reps):
            with tc.tile_pool(name="offp", bufs=1) as offp, \
                 tc.tile_pool(name="mskp", bufs=1) as mskp, \
                 tc.tile_pool(name="plp", bufs=5) as plp, \
                 tc.tile_pool(name="acp", bufs=2) as acp, \
                 tc.tile_pool(name="vcp", bufs=2) as vcp, \
                 tc.tile_pool(name="ocp", bufs=1) as ocp, \
                 tc.tile_pool(name="evp", bufs=2) as evp, \
                 tc.tile_pool(name="evq", bufs=1) as evq, \
                 tc.tile_pool(name="fin", bufs=1) as fin:

                def load_images(b):
                    """XT base copy from DRAM + 4 partition-shifted
                    SBUF->SBUF copies (big contiguous runs)."""
                    base = 2 * C * XH
                    xo = XT[:, base + 2:base + 2 + (C - 1) * XH + W]
                    xov = bass.AP(xo.tensor, xo.offset,
                                  [xo.ap[0], [XH, C], [1, W]])
                    src = x_d[b]
                    sv = bass.AP(src.tensor, src.offset,
                                 [[W, H], [H * W, C], [1, W]])
                    nc.sync.dma_start(xov, sv)
                    for d in (1, 3, 0, 4):
                        dy = d - 2
                        y0 = max(0, -dy)
                        ny = H - abs(dy)
                        dst = XT[y0:y0 + ny,
                                 d * C * XH:(d + 1) * C * XH]
                        srcv = XT[y0 + dy:y0 + dy + ny,
                                  base:base + C * XH]
                        eng = (nc.sync, nc.scalar)[d % 2]
                        eng.dma_start(dst, srcv)

                def offconv(b):
                    """3x3 conv -> OFFd[b]: PSUM chain of 9 shifted mms.

                    x rows [16ci-1, 16ci+17) staged per chunk into an
                    18-row x 130-col zero-edged c-major tile."""
                    for ci in range(NCH):
                        xs = xs_tiles[ci % 2]
                        r0 = 16 * ci - 1
                        rlo = max(0, r0)
                        rhi = min(H, r0 + 18)
                        if ci == 0:
                            nc.vector.memset(xs[:, 1:1 + W], 0.0)
                        if ci == NCH - 1:
                            nc.vector.memset(
                                xs[:, 17 * QW + 1:17 * QW + 1 + W], 0.0)
                        dsto = (rlo - r0) * QW + 1
                        dst = xs[:, dsto:dsto + (rhi - rlo - 1) * QW + W]
                        dv = bass.AP(dst.tensor, dst.offset,
                                     [dst.ap[0], [QW, rhi - rlo], [1, W]])
                        eng = (nc.sync, nc.scalar)[ci % 2]
                        eng.dma_start(dv, x_d[b, :, rlo:rhi])
                        pso = psum.tile([O, CH], f32, tag="ps", name="pso")
                        for k in range(9):
                            ki, kj = divmod(k, 3)
                            base = ki * QW + kj
                            for q4 in range(CH // 512):
                                sl = xs[:, base + q4 * 4 * QW:
                                        base + q4 * 4 * QW + 3 * QW + W]
                                rhs = bass.AP(sl.tensor, sl.offset,
                                              [sl.ap[0], [QW, 4], [1, W]])
                                nc.tensor.matmul(
                                    pso[0:18, q4 * 512:(q4 + 1) * 512],
                                    wof_sb[:, k * 18:(k + 1) * 18], rhs,
                                    start=(k == 0), stop=(k == 8))
                        oc = ocp.tile([18, CH], bf16, tag="oc", name="oc")
                        nc.scalar.activation(oc[:], pso[0:18, :],
                                             AF.Identity, bias=offb_sb[:])
                        nc.scalar.dma_start(
                            OFFd[b, :, ci * CH:(ci + 1) * CH], oc[:])

                def masks(b):
                    """offT -> vy/vx -> m81[y, (sy, k, sx, x)]."""
                    offT = offp.tile([128, 18 * W], bf16, tag="offT",
                                     name="offT")
                    src = OFFd[b]
                    nc.sync.dma_start(
                        offT[:],
                        bass.AP(src.tensor, src.offset,
                                [[W, 128], [NN, 18], [1, W]]))
                    vy = mskp.tile([128, 27 * W], bf16, tag="vy", name="vy")
                    vx = mskp.tile([128, 27 * W], bf16, tag="vx", name="vx")
                    ov = offT[:]
                    for ax, vt in ((0, vy), (1, vx)):
                        dsl = bass.AP(ov.tensor, ov.offset + ax * W,
                                      [ov.ap[0], [2 * W, 9], [1, W]])
                        for s in range(3):
                            sl = vt[:, s * 9 * W:(s + 1) * 9 * W]
                            nc.scalar.activation(sl, dsl, AF.Abs,
                                                 bias=sbias[s][:])
                            nc.scalar.activation(sl, sl, AF.Relu,
                                                 bias=1.0, scale=-1.0)
                    m81 = mskp.tile([128, 81 * W], bf16, tag="m81",
                                    name="m81")
                    vyv = vy[:]
                    vxv = vx[:]
                    for sy in range(3):
                        # out [y, (9k, (3sx, x))] = vy[sy-block k] bcast sx
                        #                         * vx[(k, sx)]
                        mo = m81[:, sy * 27 * W:(sy + 1) * 27 * W]
                        mov = bass.AP(mo.tensor, mo.offset,
                                      [mo.ap[0], [3 * W, 9], [1, 3 * W]])
                        in0 = bass.AP(vyv.tensor,
                                      vyv.offset + sy * 9 * W,
                                      [vyv.ap[0], [W, 9], [0, 3], [1, W]])
                        in1 = bass.AP(vxv.tensor, vxv.offset,
                                      [vxv.ap[0], [W, 9], [9 * W, 3],
                                       [1, W]])
                        nc.vector.tensor_tensor(mov, in0, in1, A.mult)
                    return m81

                def deform_k(b, k, m81):
                    """val_k[y, (c,x)] = sum_s m81-plane * XT-slice.

                    gpsimd sums the EARLY planes (p0..p2, available while
                    DVE is still producing) and does the final join, which
                    only the leg1 DMA consumes - so the slow gpsimd chain
                    never blocks DVE."""
                    ki, kj = divmod(k, 3)
                    planes = []
                    accA = acp.tile([128, CW], bf16, tag="accA",
                                    name="accA")
                    accB = acp.tile([128, CW], bf16, tag="accB",
                                    name="accB")
                    m81v = m81[:]
                    xtv = XT[:]
                    for si in range(9):
                        sy, sx = divmod(si, 3)
                        d = ki + sy           # 0..4 row-shift version
                        dx = kj + sx          # 0..4 col offset in xhat
                        moff = ((sy * 9 + k) * 3 + sx) * W
                        min1 = bass.AP(m81v.tensor, m81v.offset + moff,
                                       [m81v.ap[0], [0, C], [1, W]])
                        xin0 = bass.AP(xtv.tensor,
                                       xtv.offset + d * C * XH + dx,
                                       [xtv.ap[0], [XH, C], [1, W]])
                        pt = plp.tile([128, CW], bf16, tag="pt",
                                      name=f"pt{si}")
                        nc.vector.tensor_tensor(v3(pt[:]), xin0, min1,
                                                A.mult)
                        planes.append(pt)
                        if _UPTO == "prodonly":
                            continue
                        if si == 1:
                            nc.gpsimd.tensor_add(v3(accB[:]),
                                                 v3(planes[0][:]),
                                                 v3(planes[1][:]))
                        elif si == 3:
                            nc.vector.tensor_add(v3(accA[:]),
                                                 v3(planes[2][:]),
                                                 v3(planes[3][:]))
                        elif si >= 4:
                            nc.vector.tensor_add(v3(accA[:]), v3(accA[:]),
                                                 v3(planes[si][:]))
                    if _UPTO == "prodonly":
                        vt = planes[8]
                    else:
                        # join in place: leg1 reads accB, DVE never waits
                        nc.gpsimd.tensor_add(v3(accB[:]), v3(accB[:]),
                                             v3(accA[:]))
                        vt = accB
                    dd = VTd[b]
                    dst = bass.AP(dd.tensor, dd.offset + k * C * NN,
                                  [[W, H], [NN, C], [1, W]])
                    eng = (nc.sync, nc.scalar)[k % 2]
                    eng.dma_start(dst, vt[:])

                def final_chunk(b, ci, pooled_sl):
                    # gather c-major val tiles for this chunk from VTd
                    vals = []
                    for g, (kb, ng) in enumerate(KGROUPS):
                        vtile = vcp.tile([ng * C, CH], bf16, tag=f"val{g}",
                                         name=f"val{g}")
                        src = VTd[b]
                        inap = bass.AP(
                            src.tensor,
                            src.offset + kb * C * NN + ci * CH,
                            [[NN, ng * C], [1, CH]])
                        eng = (nc.sync, nc.scalar)[(ci + g) % 2]
                        eng.dma_start(vtile[:], inap)
                        vals.append(vtile)
                    ps = psum.tile([O, CH], f32, tag="ps", name="ps")
                    for g in range(3):
                        for q4 in range(CH // 512):
                            nc.tensor.matmul(
                                ps[:, q4 * 512:(q4 + 1) * 512],
                                wd_sb[g][:],
                                vals[g][:, q4 * 512:(q4 + 1) * 512],
                                start=(g == 0), stop=(g == 2))
                    col = 2 * (NCH * b + ci)
                    scr = evp.tile([O, CH], bf16, tag="scr", name="scr")
                    nc.scalar.activation(scr[:], ps[:], AF.Identity,
                                         accum_out=accp[:, col:col + 1])
                    rv = scr[:, :].rearrange("p (h w) -> p h w", w=W)
                    pw = evq.tile([O, CH // 2], bf16, tag="pw", name="pw")
                    pwv = pw[:, :].rearrange("p (h w) -> p h w", w=W // 2)
                    nc.vector.tensor_max(pwv, rv[:, :, 0:W:2],
                                         rv[:, :, 1:W:2])
                    pw3 = pw[:, :].rearrange("p (h w) -> p h w", w=W // 2)
                    mxs = evp.tile([O, CH // 4], bf16, tag="mxs",
                                   name="mxs")
                    nc.vector.tensor_max(
                        mxs[:].rearrange("p (h w) -> p h w", w=W // 2),
                        pw3[:, 0:16:2], pw3[:, 1:16:2])
                    nc.scalar.activation(scr[:], scr[:], AF.Square,
                                         accum_out=accp[:, col + 1:col + 2])
                    nc.sync.dma_start(pooled_sl, mxs[:])

                # ---------------- main schedule ----------------
                def deform_img(b, m81):
                    for k in range(9):
                        deform_k(b, k, m81)

                def finals_img(b):
                    for ci in range(NCH):
                        final_chunk(b, ci,
                                    PLd[b, :, ci * (CH // 4):
                                        (ci + 1) * (CH // 4)])

                if _UPTO != "coll":
                    load_images(0)
                    offconv(0)
                    m81_0 = masks(0)
                    deform_img(0, m81_0)
                    load_images(1)
                    offconv(1)
                    m81_1 = masks(1)
                    if _UPTO not in ("deform", "prodonly"):
                        finals_img(0)
                    deform_img(1, m81_1)
                    if _UPTO not in ("deform", "prodonly"):
                        finals_img(1)
                if _UPTO in ("deform", "finals", "prodonly"):
                    # keep the tail structure alive without the collective
                    nc.vector.memset(accp[:, 0:32], 1.0)

                # ---- BN: combine partials, allreduce across cores ----
                if _UPTO == "coll":
                    nc.vector.memset(accp[:, 0:32], 1.0)
                s12 = fin.tile([O, 2], f32, tag="s12", name="s12")
                nc.vector.tensor_add(accp[:, 0:16], accp[:, 0:16],
                                     accp[:, 16:32])
                nc.vector.tensor_add(accp[:, 0:8], accp[:, 0:8],
                                     accp[:, 8:16])
                nc.vector.tensor_add(accp[:, 0:4], accp[:, 0:4],
                                     accp[:, 4:8])
                nc.vector.tensor_add(s12[:, :], accp[:, 0:2], accp[:, 2:4])
                nc.sync.dma_start(cc_in[:], s12[:])
                nc.gpsimd.collective_compute(
                    "AllReduce", mybir.AluOpType.add,
                    replica_groups=[list(range(NCORES))],
                    ins=[cc_in.opt()], outs=[cc_out.opt()])

                s12r = fin.tile([O, 2], f32, tag="s12r", name="s12r")
                nc.sync.dma_start(s12r[:], cc_out[:])
                mr_ = fin.tile([O, 1], f32, tag="mr_", name="mr_")
                nc.vector.tensor_scalar_mul(mr_[:], s12r[:, 0:1],
                                            1.0 / NTOT)
                ex2 = fin.tile([O, 1], f32, tag="ex2", name="ex2")
                nc.vector.tensor_scalar_mul(ex2[:], s12r[:, 1:2],
                                            1.0 / NTOT)
                msq = fin.tile([O, 1], f32, tag="msq", name="msq")
                nc.vector.tensor_mul(msq[:], mr_[:], mr_[:])
                var = fin.tile([O, 1], f32, tag="var", name="var")
                nc.vector.tensor_sub(var[:], ex2[:], msq[:])
                sd = fin.tile([O, 1], f32, tag="sd", name="sd")
                nc.scalar.activation(sd[:], var[:], AF.Sqrt, bias=epsb[:])
                inv = fin.tile([O, 1], f32, tag="inv", name="inv")
                nc.vector.reciprocal(inv[:], sd[:])
                scf = fin.tile([O, 1], f32, tag="scf", name="scf")
                nc.vector.tensor_mul(scf[:], gam_sb[:], inv[:])
                tmp2 = fin.tile([O, 1], f32, tag="tmp2", name="tmp2")
                nc.vector.tensor_mul(tmp2[:], mr_[:], scf[:])
                bif = fin.tile([O, 1], f32, tag="bif", name="bif")
                nc.vector.tensor_sub(bif[:], bet_sb[:], tmp2[:])

                # ---- affine + relu on pooled maxima + store ----
                for b in range(BPC):
                    for q in range(16):
                        sl = PLd[b, :, q * 256:(q + 1) * 256]
                        plb = fin.tile([O, 256], bf16, tag="plb",
                                       name="plb")
                        nc.sync.dma_start(plb[:], sl)
                        r1 = fin.tile([O, 256], bf16, tag="r1", name="r1")
                        nc.vector.tensor_scalar(
                            r1[:], plb[:], scf[:], bif[:],
                            op0=mybir.AluOpType.mult,
                            op1=mybir.AluOpType.add)
                        po = fin.tile([O, 256], f32, tag="po", name="po")
                        nc.vector.tensor_scalar_max(po[:], r1[:], 0.0)
                        od = out_d[b]
                        nc.sync.dma_start(
                            bass.AP(od.tensor, od.offset + q * 256,
                                    [[NN // 4, O], [1, 256]]),
                            po[:, :])
    nc.compile()
    return nc


def _prep_inputs(x, offset_w, offset_b, conv_w, gamma, beta):
    """Host-side arrangement of weights into the layouts the kernel wants."""
    import ml_dtypes
    woff = np.zeros((9, C, 18), np.float32)
    for k in range(9):
        ki, kj = divmod(k, 3)
        woff[k] = offset_w[:, :, ki, kj].T
    wds = []
    for kb, ng in KGROUPS:
        blocks = []
        for kk in range(ng):
            ki, kj = divmod(kb + kk, 3)
            blocks.append(conv_w[:, :, ki, kj].T)      # [C, O]
        wds.append(np.ascontiguousarray(
            np.concatenate(blocks, axis=0)).astype(ml_dtypes.bfloat16))
    base = dict(
        woff=np.ascontiguousarray(woff).astype(ml_dtypes.bfloat16),
        wd0=wds[0], wd1=wds[1], wd2=wds[2],
        offb=offset_b.reshape(18, 1).astype(np.float32),
        gamma=gamma.reshape(O, 1).astype(np.float32),
        beta=beta.reshape(O, 1).astype(np.float32),
    )
    in_maps = []
    for ci in range(NCORES):
        m = dict(base)
        m["x_sh"] = np.ascontiguousarray(
            x[ci * BPC:(ci + 1) * BPC]).astype(ml_dtypes.bfloat16)
        in_maps.append(m)
    return in_maps


def _host_offsets(x, offset_w, offset_b):
    """offset = conv3x3(x, offset_w) + offset_b on host (|off|<1 check)."""
    xpad = np.pad(x, ((0, 0), (0, 0), (1, 1), (1, 1)))
    win = np.lib.stride_tricks.sliding_window_view(xpad, (3, 3), axis=(2, 3))
    cols = win.transpose(0, 2, 3, 1, 4, 5).reshape(B, NN, C * 9)
    w2 = offset_w.reshape(18, C * 9)
    off = cols @ w2.T.astype(np.float32)
    return off.reshape(B, H, W, 18).transpose(0, 3, 1, 2) + \
        offset_b.reshape(1, 18, 1, 1)


def _host_reference(x, offset_w, offset_b, conv_w, conv_b, gamma, beta):
    """Full numpy fallback (used only if some |offset| >= 1)."""
    off = _host_offsets(x, offset_w, offset_b).reshape(B, 9, 2, H, W)
    ki, kj = np.meshgrid(np.arange(3), np.arange(3), indexing="ij")
    base_y = (np.arange(H)[None, :, None] - 1 +
              ki.reshape(9)[:, None, None]).astype(np.float32)
    base_x = (np.arange(W)[None, None, :] - 1 +
              kj.reshape(9)[:, None, None]).astype(np.float32)
    py = off[:, :, 0] + base_y[None]
    px = off[:, :, 1] + base_x[None]
    y0 = np.floor(py).astype(np.int64)
    x0 = np.floor(px).astype(np.int64)
    wy = py - y0
    wx = px - x0
    bidx = np.arange(B)[:, None, None, None]

    def gather(iy, ix):
        valid = (iy >= 0) & (iy < H) & (ix >= 0) & (ix < W)
        v = x[bidx, :, np.clip(iy, 0, H - 1), np.clip(ix, 0, W - 1)]
        return np.where(valid[..., None], v, 0.0)

    val = (gather(y0, x0) * ((1 - wy) * (1 - wx))[..., None]
           + gather(y0, x0 + 1) * ((1 - wy) * wx)[..., None]
           + gather(y0 + 1, x0) * (wy * (1 - wx))[..., None]
           + gather(y0 + 1, x0 + 1) * (wy * wx)[..., None])
    out = np.einsum("bkhwc,ock->bohw", val, conv_w.reshape(O, C, 9),
                    optimize=True) + conv_b[None, :, None, None]
    m = out.mean(axis=(0, 2, 3), keepdims=True)
    v = out.var(axis=(0, 2, 3), keepdims=True)
    out = (out - m) / np.sqrt(v + EPS) * gamma[None, :, None, None] + \
        beta[None, :, None, None]
    out = np.maximum(out, 0.0)
    out = out.reshape(B, O, H // 2, 2, W // 2, 2).max(axis=(3, 5))
    return out.astype(np.float32)


def _get_nc(reps=1):
    key = ("nc", reps)
    if key not in _CACHE:
        _CACHE[key] = _build_nc(reps)
    return _CACHE[key]


def _run_device(in_maps, trace=False):
    from concourse import bass_utils
    nc = _get_nc()
    return bass_utils.run_bass_kernel_spmd(
        nc, in_maps, core_ids=list(range(NCORES)), trace=trace)


def kernel(x, offset_w, offset_b, mod_w, mod_b, conv_w, conv_b, gamma, beta,
           _trace=False, _return_results=False):
    x = np.asarray(x, np.float32)
    offset_w = np.asarray(offset_w, np.float32)
    offset_b = np.asarray(offset_b, np.float32)
    conv_w = np.asarray(conv_w, np.float32)
    conv_b = np.asarray(conv_b, np.float32)
    gamma = np.asarray(gamma, np.float32)
    beta = np.asarray(beta, np.float32)

    off = _host_offsets(x, offset_w, offset_b)
    if np.max(np.abs(off)) >= 0.999999 or np.min(gamma) < 0.0:
        return _host_reference(x, offset_w, offset_b, conv_w, conv_b,
                               gamma, beta)

    in_maps = _prep_inputs(x, offset_w, offset_b, conv_w, gamma, beta)
    res = _run_device(in_maps, trace=False)
    out = np.concatenate([res.results[i]["out"] for i in range(NCORES)],
                         axis=0)
    out = np.ascontiguousarray(out).astype(np.float32)
    if _return_results:
        return out, res
    return out


# revision 17
# speedup vs baseline: 1.9564x; 1.3298x over previous
"""Trainium2 Bass kernel for nn_DConv2dBlock (deformable conv block).

Pixel-major formulation (batch sharded 2 images per core across 8 cores):
  1. offset = 3x3 conv(x): PE PSUM chain of 9 shifted matmuls per chunk
     (rhs = shifted views of a zero-padded c-major image, no staging DMA).
  2. offsets permuted to pixel-major [y, (plane, x)]; triangle masks
     Lambda(dy - s) = relu(1 - |dy - s|) built by ACT; the 81 (sy, k, sx)
     mask planes m81[y, (sy,k,sx,x)] = vy * vx via 3 DVE ops per image.
  3. products in pixel-major [y, (c, x)]: for each (k,s) one DVE op
       p = m81-plane (broadcast over c via stride-0 AP) * XT-slice
     where XT[y, (dy+2, c, xhat)] holds 5 row-shifted copies of the
     x-padded image, so both shift axes are free-dim offsets and no mask
     fan-out DMA exists at all (the channel broadcast happens inside the
     DVE operand read).
  4. per (k, img): val_k = sum of 9 products; 5 adds on DVE, 3 on gpsimd.
  5. val_k dumped to DRAM (contiguous); re-read per chunk with a
     (c, y, x) gather into channel-major [(k,c), CH] tiles; PE contracts
     all 288 (k,c) rows in a 3-matmul PSUM chain per chunk.
  6. BN stats via ACT accum_out on PSUM evacuation; 2x2 maxpool inline on
     pre-BN activations (commutes with the affine since scf >= 0); 8-core
     AllReduce of (S1, S2); tiny affine+relu on pooled maxima.

The modulator branch of the reference is dead code and skipped.
conv bias cancels inside BatchNorm and is skipped.
Requires max|offset| < 1 (checked on host; falls back to a full host
computation in the measure-zero case where it does not hold).
"""

import os
import sys
import numpy as np

for _p in ("/opt/trn_rl_repo",):
    if os.path.isdir(_p) and _p not in sys.path:
        sys.path.insert(0, _p)

B, C, H, W = 16, 32, 128, 128
O = 64
NCORES = 8
BPC = B // NCORES          # images per core
NN = H * W                 # pixels per image (16384)
EPS = 1e-5
NTOT = float(B * NN)
CH = 2048                  # chunk: 16 image rows
NCH = NN // CH             # chunks per image (8)
XH = W + 4                 # padded row width for XT (132)
QW = W + 2                 # padded cols in c-major image (130)
CW = C * W                 # free size of a (c, x) plane (4096)
KGROUPS = [(0, 4), (4, 4), (8, 1)]

_CACHE = {}
_UPTO = "full"   # "deform" | "finals" | "coll" | "full"


def _build_nc(reps=1):
    import concourse.bass as bass
    import concourse.bacc as bacc
    import concourse.mybir as mybir
    from concourse import tile
    from contextlib import ExitStack

    f32 = mybir.dt.float32
    bf16 = mybir.dt.bfloat16
    AF = mybir.ActivationFunctionType
    A = mybir.AluOpType

    nc = bacc.Bacc(num_devices=NCORES)
    x_d = nc.dram_tensor("x_sh", [BPC, C, H, W], bf16, kind="ExternalInput")
    woff_d = nc.dram_tensor("woff", [9, C, 18], bf16, kind="ExternalInput")
    wd_d = [
        nc.dram_tensor("wd0", [128, O], bf16, kind="ExternalInput"),
        nc.dram_tensor("wd1", [128, O], bf16, kind="ExternalInput"),
        nc.dram_tensor("wd2", [32, O], bf16, kind="ExternalInput"),
    ]
    offb_d = nc.dram_tensor("offb", [18, 1], f32, kind="ExternalInput")
    gam_d = nc.dram_tensor("gamma", [O, 1], f32, kind="ExternalInput")
    bet_d = nc.dram_tensor("beta", [O, 1], f32, kind="ExternalInput")
    out_d = nc.dram_tensor("out", [BPC, O, H // 2, W // 2], f32,
                           kind="ExternalOutput")

    with tile.TileContext(nc) as tc, ExitStack() as ctx:
        dram = ctx.enter_context(tc.tile_pool(name="dram", bufs=1,
                                              space="DRAM"))
        OFFd = dram.tile([BPC, 18, NN], bf16)
        VTd = dram.tile([BPC, 288, NN], bf16)       # c-major val rows
        PLd = dram.tile([BPC, O, NN // 4], bf16)    # pooled maxima
        cc_in = dram.tile([O, 2], f32)
        cc_out = dram.tile([O, 2], f32)

        consts = ctx.enter_context(tc.tile_pool(name="consts", bufs=1))
        wof_sb = consts.tile([C, 9 * 18], bf16)
        nc.sync.dma_start(
            wof_sb[:],
            bass.AP(woff_d[:].tensor, 0, [[18, C], [C * 18, 9], [1, 18]]))
        wd_sb = []
        for g in range(3):
            t = consts.tile([wd_d[g].shape[0], O], bf16, tag=f"wd{g}",
                            name=f"wd{g}")
            nc.sync.dma_start(t[:], wd_d[g][:])
            wd_sb.append(t)
        offb_sb = consts.tile([18, 1], f32)
        nc.sync.dma_start(offb_sb[:], offb_d[:])
        gam_sb = consts.tile([O, 1], f32)
        nc.sync.dma_start(gam_sb[:], gam_d[:])
        bet_sb = consts.tile([O, 1], f32)
        nc.sync.dma_start(bet_sb[:], bet_d[:])
        accp = consts.tile([O, 4 * NCH], f32)
        epsb = consts.tile([O, 1], f32)
        nc.vector.memset(epsb[:], EPS)
        sbias = []
        for s in range(3):
            t = consts.tile([128, 1], f32, tag=f"sb{s}", name=f"sb{s}")
            nc.vector.memset(t[:], float(-(s - 1)))
            sbias.append(t)

        # persistent padded image; edges zeroed once, interior rewritten
        xp_pool = ctx.enter_context(tc.tile_pool(name="xp", bufs=1))
        XT = xp_pool.tile([128, 5 * C * XH], bf16)   # 5 row-shifted copies
        nc.vector.memset(XT[:], 0.0)
        # c-major conv staging: 18 rows x 130 cols, 2 slots, edges zeroed
        xs_tiles = [xp_pool.tile([C, 18 * QW], bf16, tag=f"xs{i}",
                                 name=f"xs{i}") for i in range(2)]
        for t in xs_tiles:
            nc.vector.memset(t[:], 0.0)

        psum = ctx.enter_context(tc.tile_pool(name="psum", bufs=2,
                                              space="PSUM"))

        def v3(ap):
            return ap.rearrange("p (c x) -> p c x", x=W)

        for rep in range(reps):
            with tc.tile_pool(name="offp", bufs=1) as offp, \
                 tc.tile_pool(name="mskp", bufs=1) as mskp, \
                 tc.tile_pool(name="plp", bufs=8) as plp, \
                 tc.tile_pool(name="acp", bufs=2) as acp, \
                 tc.tile_pool(name="vcp", bufs=2) as vcp, \
                 tc.tile_pool(name="ocp", bufs=1) as ocp, \
                 tc.tile_pool(name="evp", bufs=2) as evp, \
                 tc.tile_pool(name="evq", bufs=1) as evq, \
                 tc.tile_pool(name="fin", bufs=1) as fin:

                def load_images(b):
                    """XT base copy from DRAM + 4 partition-shifted
                    SBUF->SBUF copies (big contiguous runs)."""
                    base = 2 * C * XH
                    xo = XT[:, base + 2:base + 2 + (C - 1) * XH + W]
                    xov = bass.AP(xo.tensor, xo.offset,
                                  [xo.ap[0], [XH, C], [1, W]])
                    src = x_d[b]
                    sv = bass.AP(src.tensor, src.offset,
                                 [[W, H], [H * W, C], [1, W]])
                    nc.sync.dma_start(xov, sv)
                    for d in (1, 3, 0, 4):
                        dy = d - 2
                        y0 = max(0, -dy)
                        ny = H - abs(dy)
                        dst = XT[y0:y0 + ny,
                                 d * C * XH:(d + 1) * C * XH]
                        srcv = XT[y0 + dy:y0 + dy + ny,
                                  base:base + C * XH]
                        eng = (nc.sync, nc.scalar)[d % 2]
                        eng.dma_start(dst, srcv)

                def offconv(b):
                    """3x3 conv -> OFFd[b]: PSUM chain of 9 shifted mms.

                    x rows [16ci-1, 16ci+17) staged per chunk into an
                    18-row x 130-col zero-edged c-major tile."""
                    for ci in range(NCH):
                        xs = xs_tiles[ci % 2]
                        r0 = 16 * ci - 1
                        rlo = max(0, r0)
                        rhi = min(H, r0 + 18)
                        if ci == 0:
                            nc.vector.memset(xs[:, 1:1 + W], 0.0)
                        if ci == NCH - 1:
                            nc.vector.memset(
                                xs[:, 17 * QW + 1:17 * QW + 1 + W], 0.0)
                        dsto = (rlo - r0) * QW + 1
                        dst = xs[:, dsto:dsto + (rhi - rlo - 1) * QW + W]
                        dv = bass.AP(dst.tensor, dst.offset,
                                     [dst.ap[0], [QW, rhi - rlo], [1, W]])
                        eng = (nc.sync, nc.scalar)[ci % 2]
                        eng.dma_start(dv, x_d[b, :, rlo:rhi])
                        pso = psum.tile([O, CH], f32, tag="ps", name="pso")
                        for k in range(9):
                            ki, kj = divmod(k, 3)
                            base = ki * QW + kj
                            for q4 in range(CH // 512):
                                sl = xs[:, base + q4 * 4 * QW:
                                        base + q4 * 4 * QW + 3 * QW + W]
                                rhs = bass.AP(sl.tensor, sl.offset,
                                              [sl.ap[0], [QW, 4], [1, W]])
                                nc.tensor.matmul(
                                    pso[0:18, q4 * 512:(q4 + 1) * 512],
                                    wof_sb[:, k * 18:(k + 1) * 18], rhs,
                                    start=(k == 0), stop=(k == 8))
                        oc = ocp.tile([18, CH], bf16, tag="oc", name="oc")
                        nc.scalar.activation(oc[:], pso[0:18, :],
                                             AF.Identity, bias=offb_sb[:])
                        nc.scalar.dma_start(
                            OFFd[b, :, ci * CH:(ci + 1) * CH], oc[:])

                def masks(b):
                    """offT -> vy/vx -> m81[y, (sy, k, sx, x)]."""
                    offT = offp.tile([128, 18 * W], bf16, tag="offT",
                                     name="offT")
                    src = OFFd[b]
                    nc.sync.dma_start(
                        offT[:],
                        bass.AP(src.tensor, src.offset,
                                [[W, 128], [NN, 18], [1, W]]))
                    vy = mskp.tile([128, 27 * W], bf16, tag="vy", name="vy")
                    vx = mskp.tile([128, 27 * W], bf16, tag="vx", name="vx")
                    ov = offT[:]
                    for ax, vt in ((0, vy), (1, vx)):
                        dsl = bass.AP(ov.tensor, ov.offset + ax * W,
                                      [ov.ap[0], [2 * W, 9], [1, W]])
                        for s in range(3):
                            sl = vt[:, s * 9 * W:(s + 1) * 9 * W]
                            nc.scalar.activation(sl, dsl, AF.Abs,
                                                 bias=sbias[s][:])
                            nc.scalar.activation(sl, sl, AF.Relu,
                                                 bias=1.0, scale=-1.0)
                    m81 = mskp.tile([128, 81 * W], bf16, tag="m81",
                                    name="m81")
                    vyv = vy[:]
                    vxv = vx[:]
                    for sy in range(3):
                        # out [y, (9k, (3sx, x))] = vy[sy-block k] bcast sx
                        #                         * vx[(k, sx)]
                        mo = m81[:, sy * 27 * W:(sy + 1) * 27 * W]
                        mov = bass.AP(mo.tensor, mo.offset,
                                      [mo.ap[0], [3 * W, 9], [1, 3 * W]])
                        in0 = bass.AP(vyv.tensor,
                                      vyv.offset + sy * 9 * W,
                                      [vyv.ap[0], [W, 9], [0, 3], [1, W]])
                        in1 = bass.AP(vxv.tensor, vxv.offset,
                                      [vxv.ap[0], [W, 9], [9 * W, 3],
                                       [1, W]])
                        nc.vector.tensor_tensor(mov, in0, in1, A.mult)
                    return m81

                def deform_k(b, k, m81):
                    """val_k[y, (c,x)] = sum_s m81-plane * XT-slice.

                    gpsimd sums the EARLY planes (p0..p2, available while
                    DVE is still producing) and does the final join, which
                    only the leg1 DMA consumes - so the slow gpsimd chain
                    never blocks DVE."""
                    ki, kj = divmod(k, 3)
                    m81v = m81[:]
                    xtv = XT[:]
                    prods = []
                    for si in range(9):
                        sy, sx = divmod(si, 3)
                        d = ki + sy           # 0..4 row-shift version
                        dx = kj + sx          # 0..4 col offset in xhat
                        moff = ((sy * 9 + k) * 3 + sx) * W
                        min1 = bass.AP(m81v.tensor, m81v.offset + moff,
                                       [m81v.ap[0], [0, C], [1, W]])
                        xin0 = bass.AP(xtv.tensor,
                                       xtv.offset + d * C * XH + dx,
                                       [xtv.ap[0], [XH, C], [1, W]])
                        prods.append((xin0, min1))
                    # interleaved products + out-of-place add tree on a
                    # 5-slot ring; every slot's prior tenant is consumed
                    # 1-2 ops earlier, so the pipeline never stalls.
                    planes = []
                    sums = []

                    def emit_prod(i):
                        pt = plp.tile([128, CW], bf16, tag="pt",
                                      name=f"pt{i}")
                        nc.vector.tensor_tensor(v3(pt[:]), prods[i][0],
                                                prods[i][1], A.mult)
                        planes.append(pt)

                    def emit_add(a, bb):
                        st = plp.tile([128, CW], bf16, tag="pt",
                                      name="st")
                        nc.vector.tensor_add(v3(st[:]), v3(a[:]),
                                             v3(bb[:]))
                        sums.append(st)
                        return st

                    emit_prod(0)
                    emit_prod(1)
                    emit_prod(2)
                    t = emit_add(planes[0], planes[1])
                    for i in range(3, 9):
                        emit_prod(i)
                        t = emit_add(t, planes[i - 1])
                    if _UPTO == "prodonly":
                        vt = planes[8]
                    else:
                        vt = emit_add(t, planes[8])
                    dd = VTd[b]
                    dst = bass.AP(dd.tensor, dd.offset + k * C * NN,
                                  [[W, H], [NN, C], [1, W]])
                    eng = (nc.sync, nc.scalar)[k % 2]
                    eng.dma_start(dst, vt[:])

                def final_chunk(b, ci, pooled_sl):
                    # gather c-major val tiles for this chunk from VTd
                    vals = []
                    for g, (kb, ng) in enumerate(KGROUPS):
                        vtile = vcp.tile([ng * C, CH], bf16, tag=f"val{g}",
                                         name=f"val{g}")
                        src = VTd[b]
                        inap = bass.AP(
                            src.tensor,
                            src.offset + kb * C * NN + ci * CH,
                            [[NN, ng * C], [1, CH]])
                        eng = (nc.sync, nc.scalar)[(ci + g) % 2]
                        eng.dma_start(vtile[:], inap)
                        vals.append(vtile)
                    ps = psum.tile([O, CH], f32, tag="ps", name="ps")
                    for g in range(3):
                        for q4 in range(CH // 512):
                            nc.tensor.matmul(
                                ps[:, q4 * 512:(q4 + 1) * 512],
                                wd_sb[g][:],
                                vals[g][:, q4 * 512:(q4 + 1) * 512],
                                start=(g == 0), stop=(g == 2))
                    col = 2 * (NCH * b + ci)
                    scr = evp.tile([O, CH], bf16, tag="scr", name="scr")
                    nc.scalar.activation(scr[:], ps[:], AF.Identity,
                                         accum_out=accp[:, col:col + 1])
                    rv = scr[:, :].rearrange("p (h w) -> p h w", w=W)
                    pw = evq.tile([O, CH // 2], bf16, tag="pw", name="pw")
                    pwv = pw[:, :].rearrange("p (h w) -> p h w", w=W // 2)
                    nc.vector.tensor_max(pwv, rv[:, :, 0:W:2],
                                         rv[:, :, 1:W:2])
                    pw3 = pw[:, :].rearrange("p (h w) -> p h w", w=W // 2)
                    mxs = evp.tile([O, CH // 4], bf16, tag="mxs",
                                   name="mxs")
                    nc.vector.tensor_max(
                        mxs[:].rearrange("p (h w) -> p h w", w=W // 2),
                        pw3[:, 0:16:2], pw3[:, 1:16:2])
                    nc.scalar.activation(scr[:], scr[:], AF.Square,
                                         accum_out=accp[:, col + 1:col + 2])
                    nc.sync.dma_start(pooled_sl, mxs[:])

                # ---------------- main schedule ----------------
                def deform_img(b, m81):
                    for k in range(9):
                        deform_k(b, k, m81)

                def finals_img(b):
                    for ci in range(NCH):
                        final_chunk(b, ci,
                                    PLd[b, :, ci * (CH // 4):
                                        (ci + 1) * (CH // 4)])

                if _UPTO != "coll":
                    load_images(0)
                    offconv(0)
                    m81_0 = masks(0)
                    deform_img(0, m81_0)
                    load_images(1)
                    offconv(1)
                    m81_1 = masks(1)
                    if _UPTO not in ("deform", "prodonly"):
                        finals_img(0)
                    deform_img(1, m81_1)
                    if _UPTO not in ("deform", "prodonly"):
                        finals_img(1)
                if _UPTO in ("deform", "finals", "prodonly"):
                    # keep the tail structure alive without the collective
                    nc.vector.memset(accp[:, 0:32], 1.0)

                # ---- BN: combine partials, allreduce across cores ----
                if _UPTO == "coll":
                    nc.vector.memset(accp[:, 0:32], 1.0)
                s12 = fin.tile([O, 2], f32, tag="s12", name="s12")
                nc.vector.tensor_add(accp[:, 0:16], accp[:, 0:16],
                                     accp[:, 16:32])
                nc.vector.tensor_add(accp[:, 0:8], accp[:, 0:8],
                                     accp[:, 8:16])
                nc.vector.tensor_add(accp[:, 0:4], accp[:, 0:4],
                                     accp[:, 4:8])
                nc.vector.tensor_add(s12[:, :], accp[:, 0:2], accp[:, 2:4])
                nc.sync.dma_start(cc_in[:], s12[:])
                nc.gpsimd.collective_compute(
                    "AllReduce", mybir.AluOpType.add,
                    replica_groups=[list(range(NCORES))],
                    ins=[cc_in.opt()], outs=[cc_out.opt()])

                s12r = fin.tile([O, 2], f32, tag="s12r", name="s12r")
                nc.sync.dma_start(s12r[:], cc_out[:])
                mr_ = fin.tile([O, 1], f32, tag="mr_", name="mr_")
                nc.vector.tensor_scalar_mul(mr_[:], s12r[:, 0:1],
                                            1.0 / NTOT)
                ex2 = fin.tile([O, 1], f32, tag="ex2", name="ex2")
                nc.vector.tensor_scalar_mul(ex2[:], s12r[:, 1:2],
                                            1.0 / NTOT)
                msq = fin.tile([O, 1], f32, tag="msq", name="msq")
                nc.vector.tensor_mul(msq[:], mr_[:], mr_[:])
                var = fin.tile([O, 1], f32, tag="var", name="var")
                nc.vector.tensor_sub(var[:], ex2[:], msq[:])
                sd = fin.tile([O, 1], f32, tag="sd", name="sd")
                nc.scalar.activation(sd[:], var[:], AF.Sqrt, bias=epsb[:])
                inv = fin.tile([O, 1], f32, tag="inv", name="inv")
                nc.vector.reciprocal(inv[:], sd[:])
                scf = fin.tile([O, 1], f32, tag="scf", name="scf")
                nc.vector.tensor_mul(scf[:], gam_sb[:], inv[:])
                tmp2 = fin.tile([O, 1], f32, tag="tmp2", name="tmp2")
                nc.vector.tensor_mul(tmp2[:], mr_[:], scf[:])
                bif = fin.tile([O, 1], f32, tag="bif", name="bif")
                nc.vector.tensor_sub(bif[:], bet_sb[:], tmp2[:])

                # ---- affine + relu on pooled maxima + store ----
                for b in range(BPC):
                    for q in range(16):
                        sl = PLd[b, :, q * 256:(q + 1) * 256]
                        plb = fin.tile([O, 256], bf16, tag="plb",
                                       name="plb")
                        nc.sync.dma_start(plb[:], sl)
                        r1 = fin.tile([O, 256], bf16, tag="r1", name="r1")
                        nc.vector.tensor_scalar(
                            r1[:], plb[:], scf[:], bif[:],
                            op0=mybir.AluOpType.mult,
                            op1=mybir.AluOpType.add)
                        po = fin.tile([O, 256], f32, tag="po", name="po")
                        nc.vector.tensor_scalar_max(po[:], r1[:], 0.0)
                        od = out_d[b]
                        nc.sync.dma_start(
                            bass.AP(od.tensor, od.offset + q * 256,
                                    [[NN // 4, O], [1, 256]]),
                            po[:, :])
    nc.compile()
    return nc


def _prep_inputs(x, offset_w, offset_b, conv_w, gamma, beta):
    """Host-side arrangement of weights into the layouts the kernel wants."""
    import ml_dtypes
    woff = np.zeros((9, C, 18), np.float32)
    for k in range(9):
        ki, kj = divmod(k, 3)
        woff[k] = offset_w[:, :, ki, kj].T
    wds = []
    for kb, ng in KGROUPS:
        blocks = []
        for kk in range(ng):
            ki, kj = divmod(kb + kk, 3)
            blocks.append(conv_w[:, :, ki, kj].T)      # [C, O]
        wds.append(np.ascontiguousarray(
            np.concatenate(blocks, axis=0)).astype(ml_dtypes.bfloat16))
    base = dict(
        woff=np.ascontiguousarray(woff).astype(ml_dtypes.bfloat16),
        wd0=wds[0], wd1=wds[1], wd2=wds[2],
        offb=offset_b.reshape(18, 1).astype(np.float32),
        gamma=gamma.reshape(O, 1).astype(np.float32),
        beta=beta.reshape(O, 1).astype(np.float32),
    )
    in_maps = []
    for ci in range(NCORES):
        m = dict(base)
        m["x_sh"] = np.ascontiguousarray(
            x[ci * BPC:(ci + 1) * BPC]).astype(ml_dtypes.bfloat16)
        in_maps.append(m)
    return in_maps


def _host_offsets(x, offset_w, offset_b):
    """offset = conv3x3(x, offset_w) + offset_b on host (|off|<1 check)."""
    xpad = np.pad(x, ((0, 0), (0, 0), (1, 1), (1, 1)))
    win = np.lib.stride_tricks.sliding_window_view(xpad, (3, 3), axis=(2, 3))
    cols = win.transpose(0, 2, 3, 1, 4, 5).reshape(B, NN, C * 9)
    w2 = offset_w.reshape(18, C * 9)
    off = cols @ w2.T.astype(np.float32)
    return off.reshape(B, H, W, 18).transpose(0, 3, 1, 2) + \
        offset_b.reshape(1, 18, 1, 1)


def _host_reference(x, offset_w, offset_b, conv_w, conv_b, gamma, beta):
    """Full numpy fallback (used only if some |offset| >= 1)."""
    off = _host_offsets(x, offset_w, offset_b).reshape(B, 9, 2, H, W)
    ki, kj = np.meshgrid(np.arange(3), np.arange(3), indexing="ij")
    base_y = (np.arange(H)[None, :, None] - 1 +
              ki.reshape(9)[:, None, None]).astype(np.float32)
    base_x = (np.arange(W)[None, None, :] - 1 +
              kj.reshape(9)[:, None, None]).astype(np.float32)
    py = off[:, :, 0] + base_y[None]
    px = off[:, :, 1] + base_x[None]
    y0 = np.floor(py).astype(np.int64)
    x0 = np.floor(px).astype(np.int64)
    wy = py - y0
    wx = px - x0
    bidx = np.arange(B)[:, None, None, None]

    def gather(iy, ix):
        valid = (iy >= 0) & (iy < H) & (ix >= 0) & (ix < W)
        v = x[bidx, :, np.clip(iy, 0, H - 1), np.clip(ix, 0, W - 1)]
        return np.where(valid[..., None], v, 0.0)

    val = (gather(y0, x0) * ((1 - wy) * (1 - wx))[..., None]
           + gather(y0, x0 + 1) * ((1 - wy) * wx)[..., None]
           + gather(y0 + 1, x0) * (wy * (1 - wx))[..., None]
           + gather(y0 + 1, x0 + 1) * (wy * wx)[..., None])
    out = np.einsum("bkhwc,ock->bohw", val, conv_w.reshape(O, C, 9),
                    optimize=True) + conv_b[None, :, None, None]
    m = out.mean(axis=(0, 2, 3), keepdims=True)
    v = out.var(axis=(0, 2, 3), keepdims=True)
    out = (out - m) / np.sqrt(v + EPS) * gamma[None, :, None, None] + \
        beta[None, :, None, None]
    out = np.maximum(out, 0.0)
    out = out.reshape(B, O, H // 2, 2, W // 2, 2).max(axis=(3, 5))
    return out.astype(np.float32)


def _get_nc(reps=1):
    key = ("nc", reps)
    if key not in _CACHE:
        _CACHE[key] = _build_nc(reps)
    return _CACHE[key]


def _run_device(in_maps, trace=False):
    from concourse import bass_utils
    nc = _get_nc()
    return bass_utils.run_bass_kernel_spmd(
        nc, in_maps, core_ids=list(range(NCORES)), trace=trace)


def kernel(x, offset_w, offset_b, mod_w, mod_b, conv_w, conv_b, gamma, beta,
           _trace=False, _return_results=False):
    x = np.asarray(x, np.float32)
    offset_w = np.asarray(offset_w, np.float32)
    offset_b = np.asarray(offset_b, np.float32)
    conv_w = np.asarray(conv_w, np.float32)
    conv_b = np.asarray(conv_b, np.float32)
    gamma = np.asarray(gamma, np.float32)
    beta = np.asarray(beta, np.float32)

    off = _host_offsets(x, offset_w, offset_b)
    if np.max(np.abs(off)) >= 0.999999 or np.min(gamma) < 0.0:
        return _host_reference(x, offset_w, offset_b, conv_w, conv_b,
                               gamma, beta)

    in_maps = _prep_inputs(x, offset_w, offset_b, conv_w, gamma, beta)
    res = _run_device(in_maps, trace=False)
    out = np.concatenate([res.results[i]["out"] for i in range(NCORES)],
                         axis=0)
    out = np.ascontiguousarray(out).astype(np.float32)
    if _return_results:
        return out, res
    return out
